# revision 1
# baseline (speedup 1.0000x reference)
"""BlockDiffusionDecoder (mBART-style 2-layer decoder + BD3LM self-attn mask)
on 8 Trainium2 NeuronCores.

Sharding: cores (2b, 2b+1) own batch element b (B=4 -> 8 cores).  Within a
pair, tensor-parallel over heads (8 of 16) and d_ff (2048 of 4096), with a
pair AllReduce after the o-projections and fc2.  The LM head is sharded over
vocab 8 ways (padded 32768 = 8 x 4096) after an AllGather of final hidden
states across the {even} / {odd} core groups.

Layouts: activations live in SBUF as [128 tokens, tile, feature]; transposed
copies ([feature-tile, token]) are built with PE transposes.  All matmuls run
in bf16 (full PE rate); residual stream / LN / softmax stats stay fp32.
Weights are shipped from host pre-tiled and pre-cast to bf16.
"""
import sys

if "/opt/trn_rl_repo" not in sys.path:
    sys.path.insert(0, "/opt/trn_rl_repo")

import contextlib

import ml_dtypes
import numpy as np

import concourse.bass as bass
import concourse.bacc as bacc
import concourse.tile as tile
from concourse import mybir
from concourse.bass_utils import run_bass_kernel_spmd
from concourse.masks import make_identity

P = 128
B, D, H, NL, DFF, V, S = 4, 1024, 16, 2, 4096, 32000, 128
T = 1024
HD = D // H          # 64
BLK = 4
VP = 32768           # padded vocab (32001 -> 8*4096)
VSH = VP // 8        # vocab shard per core
NT = T // P          # 8 token tiles
ND = D // P          # 8 feature tiles
EMB_SCALE = 32.0     # sqrt(D)
FMIN = float(np.finfo(np.float32).min)
BF = ml_dtypes.bfloat16

f32 = mybir.dt.float32
bf16 = mybir.dt.bfloat16
i32 = mybir.dt.int32
AF = mybir.ActivationFunctionType
ALU = mybir.AluOpType
AX = mybir.AxisListType


def _rhs_tile(w_t: np.ndarray, nchunk: int) -> np.ndarray:
    """[d_in, d_out] -> [n_chunks, 128, k_tiles, nchunk] bf16, so the DMA of
    one n-chunk is contiguous per partition (k-major, n-minor)."""
    d_in, d_out = w_t.shape
    kt = d_in // P
    nc_ = d_out // nchunk
    return np.ascontiguousarray(
        w_t.reshape(kt, P, nc_, nchunk).transpose(2, 1, 0, 3).astype(BF))


def host_prepare(inputs: dict, hsh: int, dsh: int, vsh: int):
    """Build per-core input maps. hsh: heads/core, dsh: d_ff/core."""
    tp = 16 // hsh
    ids = np.asarray(inputs["input_ids"])
    enc = np.asarray(inputs["enc_hidden"], dtype=np.float32)
    emask = np.asarray(inputs["enc_mask"])
    emb = np.ascontiguousarray(np.asarray(inputs["embed_tokens"], np.float32))
    pos = np.ascontiguousarray(np.asarray(inputs["pos_embed"], np.float32))
    attn_w = np.asarray(inputs["attn_w"], np.float32)
    attn_b = np.asarray(inputs["attn_b"], np.float32)
    ln_w = np.asarray(inputs["ln_w"], np.float32)
    ln_b = np.asarray(inputs["ln_b"], np.float32)
    fc1_w = np.asarray(inputs["fc1_w"], np.float32)
    fc1_b = np.asarray(inputs["fc1_b"], np.float32)
    fc2_w = np.asarray(inputs["fc2_w"], np.float32)
    fc2_b = np.asarray(inputs["fc2_b"], np.float32)
    lm_w = np.asarray(inputs["lm_head_w"], np.float32)

    lm_pad = np.zeros((VP, D), np.float32)
    lm_pad[: V + 1] = lm_w
    lm_t = lm_pad.T  # [D, VP]

    n_cores = 8 if tp == 2 else 1
    maps = []
    for c in range(n_cores):
        b_ = c // tp
        j = c % tp
        hs = slice(j * hsh * HD, (j + 1) * hsh * HD)
        ds_ = slice(j * dsh, (j + 1) * dsh)
        vs_ = slice(c * vsh, (c + 1) * vsh) if tp == 2 else slice(0, vsh)
        m = {
            "ids": ids[b_].reshape(T, 1).astype(np.int32),
            "emb": emb,
            "pos": pos,
            "encT": np.ascontiguousarray(enc[b_].T.astype(BF)),   # [D, S]
            "cmask": ((1.0 - emask[b_].astype(np.float32)) * FMIN)
            .reshape(1, S),
            "lnemb": np.stack([np.asarray(inputs["ln_emb_s"], np.float32),
                               np.asarray(inputs["ln_emb_b"], np.float32)]),
            "lnfin": np.stack([np.asarray(inputs["final_ln_s"], np.float32),
                               np.asarray(inputs["final_ln_b"], np.float32)]),
            "lnw": ln_w, "lnb": ln_b,
            "wlm": np.ascontiguousarray(
                lm_t[:, vs_].reshape(ND, P, vsh).transpose(1, 0, 2)
                .astype(BF)),
        }
        for l in range(NL):
            for a, tag in ((0, "s"), (1, "c")):
                wq, wk, wv, wo = attn_w[l, a]
                bq, bk, bv, bo = attn_b[l, a]
                m[f"wq{tag}{l}"] = _rhs_tile(wq.T[:, hs], P)
                m[f"wk{tag}{l}"] = _rhs_tile(wk.T[:, hs], P)
                m[f"wv{tag}{l}"] = _rhs_tile(wv.T[:, hs], hsh * HD)
                m[f"wo{tag}{l}"] = _rhs_tile(wo.T[hs, :], D // 2)
                mh = hsh * HD // P
                m[f"bq{tag}{l}"] = np.ascontiguousarray(
                    bq[hs].reshape(mh, P).T)
                m[f"bk{tag}{l}"] = np.ascontiguousarray(
                    bk[hs].reshape(mh, P).T)
                m[f"bv{tag}{l}"] = bv[hs].reshape(1, hsh * HD).copy()
                m[f"bo{tag}{l}"] = (bo / tp).reshape(1, D).copy()
            m[f"wf1{l}"] = _rhs_tile(fc1_w[l].T[:, ds_], P)
            m[f"bf1{l}"] = np.ascontiguousarray(
                fc1_b[l][ds_].reshape(dsh // P, P).T)
            m[f"wf2{l}"] = np.ascontiguousarray(
                fc2_w[l].T[ds_, :].reshape(dsh // P, P, 2, 512)
                .transpose(2, 1, 0, 3).astype(BF))  # [n2, p, kk, 512]
            m[f"bf2{l}"] = (fc2_b[l] / tp).reshape(1, D).copy()
        maps.append(m)
    return maps


def _mask_consts():
    i = np.arange(P)
    diag = np.where((i[:, None] // BLK) == (i[None, :] // BLK), 0.0, FMIN)
    tri_s = np.where((i[:, None] // BLK) > (i[None, :] // BLK), 0.0, FMIN)
    tri_i = np.where((i[:, None] // BLK) >= (i[None, :] // BLK), 0.0, FMIN)
    return (diag.astype(np.float32), tri_s.astype(np.float32),
            tri_i.astype(np.float32))


def build_nc(hsh=8, dsh=2048, vsh=VSH, nb_lm=4, collectives=True,
             gelu=AF.Gelu_apprx_tanh):
    tp = 16 // hsh
    MH = hsh * HD // P        # d_out tiles for q/k/v shard
    KO = MH                   # k-tiles for o-proj lhs
    NDC = dsh // 512          # dff chunks
    nc = bacc.Bacc(num_devices=8 if collectives else None, trn_type="TRN2")

    ids_d = nc.dram_tensor("ids", [T, 1], i32, kind="ExternalInput")
    emb_d = nc.dram_tensor("emb", [V + 1, D], f32, kind="ExternalInput")
    pos_d = nc.dram_tensor("pos", [T, D], f32, kind="ExternalInput")
    encT_d = nc.dram_tensor("encT", [D, S], bf16, kind="ExternalInput")
    cmask_d = nc.dram_tensor("cmask", [1, S], f32, kind="ExternalInput")
    lnemb_d = nc.dram_tensor("lnemb", [2, D], f32, kind="ExternalInput")
    lnfin_d = nc.dram_tensor("lnfin", [2, D], f32, kind="ExternalInput")
    lnw_d = nc.dram_tensor("lnw", [NL, 3, D], f32, kind="ExternalInput")
    lnb_d = nc.dram_tensor("lnb", [NL, 3, D], f32, kind="ExternalInput")
    wlm_d = nc.dram_tensor("wlm", [P, ND, vsh], bf16, kind="ExternalInput")
    wd, bd = {}, {}
    for l in range(NL):
        for tg in ("s", "c"):
            wd[f"wq{tg}{l}"] = nc.dram_tensor(
                f"wq{tg}{l}", [MH, P, ND, P], bf16, kind="ExternalInput")
            wd[f"wk{tg}{l}"] = nc.dram_tensor(
                f"wk{tg}{l}", [MH, P, ND, P], bf16, kind="ExternalInput")
            wd[f"wv{tg}{l}"] = nc.dram_tensor(
                f"wv{tg}{l}", [1, P, ND, hsh * HD], bf16,
                kind="ExternalInput")
            wd[f"wo{tg}{l}"] = nc.dram_tensor(
                f"wo{tg}{l}", [2, P, KO, D // 2], bf16, kind="ExternalInput")
            bd[f"bq{tg}{l}"] = nc.dram_tensor(
                f"bq{tg}{l}", [P, MH], f32, kind="ExternalInput")
            bd[f"bk{tg}{l}"] = nc.dram_tensor(
                f"bk{tg}{l}", [P, MH], f32, kind="ExternalInput")
            bd[f"bv{tg}{l}"] = nc.dram_tensor(
                f"bv{tg}{l}", [1, hsh * HD], f32, kind="ExternalInput")
            bd[f"bo{tg}{l}"] = nc.dram_tensor(
                f"bo{tg}{l}", [1, D], f32, kind="ExternalInput")
        wd[f"wf1{l}"] = nc.dram_tensor(
            f"wf1{l}", [dsh // P, P, ND, P], bf16, kind="ExternalInput")
        bd[f"bf1{l}"] = nc.dram_tensor(
            f"bf1{l}", [P, dsh // P], f32, kind="ExternalInput")
        wd[f"wf2{l}"] = nc.dram_tensor(
            f"wf2{l}", [2, P, dsh // P, 512], bf16, kind="ExternalInput")
        bd[f"bf2{l}"] = nc.dram_tensor(
            f"bf2{l}", [1, D], f32, kind="ExternalInput")
    out_d = nc.dram_tensor("out", [nb_lm * T, vsh], f32,
                           kind="ExternalOutput")

    mdiag_np, mtris_np, mtrii_np = _mask_consts()
    mdiag_d = nc.inline_tensor(mdiag_np, "mdiag")
    mtris_d = nc.inline_tensor(mtris_np, "mtris")
    mtrii_d = nc.inline_tensor(mtrii_np, "mtrii")

    PAIRS = [[0, 1], [2, 3], [4, 5], [6, 7]]
    EVENODD = [[0, 2, 4, 6], [1, 3, 5, 7]]

    def bcast(ap_1d, p=P):
        return bass.AP(tensor=ap_1d.tensor, offset=ap_1d.offset,
                       ap=[[0, p]] + list(ap_1d.ap))

    with tile.TileContext(nc) as tc:
        gctx = contextlib.ExitStack()
        with gctx:
            consts = gctx.enter_context(tc.tile_pool(name="consts", bufs=1))
            small = gctx.enter_context(tc.tile_pool(name="small", bufs=4))
            sp = gctx.enter_context(tc.tile_pool(name="sp", bufs=2))
            xp = gctx.enter_context(tc.tile_pool(name="xp", bufs=3))
            xb = gctx.enter_context(tc.tile_pool(name="xb", bufs=2))
            dram = gctx.enter_context(
                tc.tile_pool(name="dram", bufs=1, space="DRAM"))
            ps_a = gctx.enter_context(
                tc.tile_pool(name="ps_a", bufs=4, space="PSUM"))
            ps_av = gctx.enter_context(
                tc.tile_pool(name="ps_av", bufs=2, space="PSUM"))
            ps_tr = gctx.enter_context(
                tc.tile_pool(name="ps_tr", bufs=2, space="PSUM"))

            ident = consts.tile([P, P], bf16)
            make_identity(nc, ident[:])
            eps_t = consts.tile([P, 1], f32)
            nc.vector.memset(eps_t[:], 1e-5)
            mdiag = consts.tile([P, P], f32)
            nc.sync.dma_start(out=mdiag[:], in_=mdiag_d[:])
            mtris = consts.tile([P, P], f32)
            nc.sync.dma_start(out=mtris[:], in_=mtris_d[:])
            mtrii = consts.tile([P, P], f32)
            nc.sync.dma_start(out=mtrii[:], in_=mtrii_d[:])
            cmask_b = consts.tile([P, S], f32)
            nc.sync.dma_start(out=cmask_b[:], in_=bcast(cmask_d[0]))

            ccr_in = [dram.tile([T // 2, D], bf16, name=f"ccr_in{i}")
                      for i in range(2)]
            ccr_out = ([dram.tile([T // 2, D], bf16, name=f"ccr_out{i}")
                        for i in range(2)] if collectives else ccr_in)
            ccf_in = [dram.tile([T // 2, D], bf16, name=f"ccf_in{i}")
                      for i in range(2)]
            ccf_out = ([dram.tile([nb_lm, T // 2, D], bf16,
                                  name=f"ccf_out{i}") for i in range(2)]
                       if collectives else ccf_in)

            def ln_tile(src_ap, dst_ap, s_b, b_b):
                st = small.tile([P, 2, 6], f32, name="lnstats")
                nc.vector.bn_stats(out=st[:, 0, :], in_=src_ap[:, 0:512])
                nc.vector.bn_stats(out=st[:, 1, :], in_=src_ap[:, 512:1024])
                mv = small.tile([P, 2], f32, name="lnmv")
                nc.vector.bn_aggr(out=mv[:], in_=st[:])
                rstd = small.tile([P, 1], f32, name="lnrstd")
                nc.scalar.activation(out=rstd[:], in_=mv[:, 1:2],
                                     func=AF.Sqrt, bias=eps_t[:])
                nc.vector.reciprocal(out=rstd[:], in_=rstd[:])
                tmp = xp.tile([P, D], f32, name="xrow")
                nc.vector.tensor_scalar(out=tmp[:], in0=src_ap,
                                        scalar1=mv[:, 0:1], scalar2=rstd[:],
                                        op0=ALU.subtract, op1=ALU.mult)
                nc.vector.tensor_tensor(out=tmp[:], in0=tmp[:], in1=s_b,
                                        op=ALU.mult)
                nc.vector.tensor_tensor(out=dst_ap, in0=tmp[:], in1=b_b,
                                        op=ALU.add)

            # ---------------- stack phase ----------------
            sctx = contextlib.ExitStack()
            with sctx:
                pers = sctx.enter_context(tc.tile_pool(name="pers", bufs=1))
                wp = sctx.enter_context(tc.tile_pool(name="wp", bufs=2))
                lnp = sctx.enter_context(tc.tile_pool(name="lnp", bufs=1))

                h = pers.tile([P, NT, D], f32)
                encT = pers.tile([P, ND, S], bf16)
                nc.sync.dma_start(
                    out=encT[:],
                    in_=encT_d.rearrange("(k p) s -> p k s", p=P))

                def ln_consts(s_src, b_src):
                    s_b = lnp.tile([P, D], bf16, name="ln_s")
                    nc.gpsimd.dma_start(out=s_b[:], in_=bcast(s_src))
                    b_b = lnp.tile([P, D], bf16, name="ln_b")
                    nc.gpsimd.dma_start(out=b_b[:], in_=bcast(b_src))
                    return s_b, b_b

                def ln_to_xT(dst_xT, s_b, b_b):
                    """x = LN(h) (bf16) then xT[:, k, tt*128:] = T(x)."""
                    for tt in range(NT):
                        xt_ = xb.tile([P, D], bf16, name="xbrow")
                        ln_tile(h[:, tt, :], xt_[:], s_b[:], b_b[:])
                        for k in range(ND):
                            tp_ = ps_tr.tile([P, P], bf16, name="trps")
                            nc.tensor.transpose(
                                tp_[:], xt_[:, k * P:(k + 1) * P], ident[:])
                            if k % 2 == 0:
                                nc.scalar.activation(
                                    out=dst_xT[:, k, tt * P:(tt + 1) * P],
                                    in_=tp_[:], func=AF.Copy)
                            else:
                                nc.vector.tensor_copy(
                                    out=dst_xT[:, k, tt * P:(tt + 1) * P],
                                    in_=tp_[:])

                def proj_qk(dst, w_key, b_sb, scale, src_xT):
                    """dst[pd, m, t] = (xT.T @ W)^T with bias (+opt scale)."""
                    for m in range(MH):
                        wch = wp.tile([P, ND, P], bf16, name="wch")
                        nc.sync.dma_start(out=wch[:], in_=wd[w_key][m])
                        for half in range(2):
                            psq = ps_a.tile([P, 512], f32, name="psq")
                            for k in range(ND):
                                nc.tensor.matmul(
                                    out=psq[:],
                                    lhsT=wch[:, k, :],
                                    rhs=src_xT[:, k,
                                               half * 512:(half + 1) * 512],
                                    start=(k == 0), stop=(k == ND - 1))
                            if scale is None:
                                nc.vector.tensor_scalar(
                                    out=dst[:, m, half * 512:(half + 1) * 512],
                                    in0=psq[:], scalar1=b_sb[:, m:m + 1],
                                    scalar2=None, op0=ALU.add)
                            else:
                                nc.vector.tensor_scalar(
                                    out=dst[:, m, half * 512:(half + 1) * 512],
                                    in0=psq[:], scalar1=b_sb[:, m:m + 1],
                                    scalar2=scale, op0=ALU.add, op1=ALU.mult)

                def ar_h_update(rh):
                    """DMA back one token-half of a reduced delta and
                    accumulate into h (bias was folded pre-AllReduce)."""
                    for tl in range(4):
                        tt = 4 * rh + tl
                        dtile = xb.tile([P, D], bf16, name="xbrow")
                        nc.sync.dma_start(
                            out=dtile[:],
                            in_=ccr_out[rh][tl * P:(tl + 1) * P, :])
                        nc.vector.tensor_tensor(out=h[:, tt, :],
                                                in0=h[:, tt, :],
                                                in1=dtile[:], op=ALU.add)

                def oproj_ar_update(src_oT, wo_key, bo_key):
                    """o-proj partial -> AllReduce per token half (bf16,
                    pipelined with the next half) -> h update."""
                    bo_b = lnp.tile([P, D], bf16, name="bo_b")
                    nc.gpsimd.dma_start(out=bo_b[:], in_=bcast(bd[bo_key][0]))
                    wchs = []
                    for half in range(2):
                        wch = wp.tile([P, KO, 512], bf16, name="wch")
                        nc.sync.dma_start(out=wch[:], in_=wd[wo_key][half])
                        wchs.append(wch)
                    for rh in range(2):
                        for tl in range(4):
                            tt = 4 * rh + tl
                            for half in range(2):
                                pso = ps_a.tile([P, 512], f32, name="psq")
                                for k in range(KO):
                                    nc.tensor.matmul(
                                        out=pso[:],
                                        lhsT=src_oT[:, k,
                                                    tt * P:(tt + 1) * P],
                                        rhs=wchs[half][:, k, :],
                                        start=(k == 0), stop=(k == KO - 1))
                                ev = xb.tile([P, 512], bf16, name="evb")
                                nc.vector.tensor_tensor(
                                    out=ev[:], in0=pso[:],
                                    in1=bo_b[:, half * 512:(half + 1) * 512],
                                    op=ALU.add)
                                nc.sync.dma_start(
                                    out=ccr_in[rh][tl * P:(tl + 1) * P,
                                                   half * 512:
                                                   (half + 1) * 512],
                                    in_=ev[:])
                        if collectives:
                            nc.gpsimd.collective_compute(
                                "AllReduce", ALU.add, replica_groups=PAIRS,
                                ins=[ccr_in[rh][:]], outs=[ccr_out[rh][:]])
                        ar_h_update(rh)

                # ---- embed + emb LN ----
                lnes, lneb = ln_consts(lnemb_d[0], lnemb_d[1])
                for tt in range(NT):
                    idt = small.tile([P, 1], i32, name="idt")
                    nc.sync.dma_start(out=idt[:],
                                      in_=ids_d[tt * P:(tt + 1) * P])
                    g = xp.tile([P, D], f32, name="xrow")
                    nc.gpsimd.indirect_dma_start(
                        out=g[:], out_offset=None, in_=emb_d[:],
                        in_offset=bass.IndirectOffsetOnAxis(
                            ap=idt[:, :1], axis=0))
                    pt = xp.tile([P, D], f32, name="xrow")
                    nc.sync.dma_start(out=pt[:],
                                      in_=pos_d[tt * P:(tt + 1) * P])
                    nc.vector.tensor_scalar(out=g[:], in0=g[:],
                                            scalar1=EMB_SCALE, scalar2=None,
                                            op0=ALU.mult)
                    nc.vector.tensor_tensor(out=h[:, tt, :], in0=g[:],
                                            in1=pt[:], op=ALU.add)
                    ln_tile(h[:, tt, :], h[:, tt, :], lnes[:], lneb[:])

                for l in range(NL):
                    # ======== self attention ========
                    lns, lnbb = ln_consts(lnw_d[l, 0], lnb_d[l, 0])
                    xT = pers.tile([P, ND, T], bf16, name="xT", tag="xT")
                    ln_to_xT(xT, lns, lnbb)

                    qT = pers.tile([P, MH, T], bf16, name="qT", tag="qT")
                    kT = pers.tile([P, MH, T], bf16, name="kT", tag="kT")
                    vv = pers.tile([P, NT, hsh * HD], bf16, name="vv",
                                   tag="vv")
                    bq_sb = small.tile([P, MH], f32, name="bq")
                    nc.sync.dma_start(out=bq_sb[:], in_=bd[f"bqs{l}"][:])
                    bk_sb = small.tile([P, MH], f32, name="bk")
                    nc.sync.dma_start(out=bk_sb[:], in_=bd[f"bks{l}"][:])
                    bv_b = lnp.tile([P, hsh * HD], bf16, name="bv_b")
                    nc.gpsimd.dma_start(out=bv_b[:],
                                        in_=bcast(bd[f"bvs{l}"][0]))

                    proj_qk(qT, f"wqs{l}", bq_sb, 0.125, xT)
                    proj_qk(kT, f"wks{l}", bk_sb, None, xT)
                    wch_v = wp.tile([P, ND, hsh * HD], bf16, name="wch")
                    nc.sync.dma_start(out=wch_v[:], in_=wd[f"wvs{l}"][0])
                    for tt in range(NT):
                        psv = ps_a.tile([P, 512], f32, name="psq")
                        for k in range(ND):
                            nc.tensor.matmul(
                                out=psv[:],
                                lhsT=xT[:, k, tt * P:(tt + 1) * P],
                                rhs=wch_v[:, k, :],
                                start=(k == 0), stop=(k == ND - 1))
                        nc.vector.tensor_tensor(out=vv[:, tt, :], in0=psv[:],
                                                in1=bv_b[:], op=ALU.add)

                    # scores + softmax + AV per head
                    oT = pers.tile([P, KO, T], bf16, name="oT", tag="oT")
                    for hl in range(hsh):
                        prow = slice((hl % 2) * 64, (hl % 2) * 64 + 64)
                        mq = hl // 2
                        ptile = sp.tile([P, 5, T], bf16, name="ptile")
                        for g in range(NT):
                            width = (g + 2) * P if g < 4 else (g - 3) * P
                            ssb = sp.tile([P, 640], f32, name="srow")
                            qst = qT[prow, mq, g * P:(g + 1) * P]
                            if g < 4:
                                psd = ps_a.tile([P, 512], f32, name="psq")
                                nc.tensor.matmul(
                                    out=psd[:, 0:P], lhsT=qst,
                                    rhs=kT[prow, mq, g * P:(g + 1) * P],
                                    start=True, stop=True)
                                pss = ps_a.tile([P, 512], f32, name="psq")
                                nc.tensor.matmul(
                                    out=pss[:, 0:width - P], lhsT=qst,
                                    rhs=kT[prow, mq, 512:512 + width - P],
                                    start=True, stop=True)
                                nc.vector.tensor_tensor(
                                    out=ssb[:, 0:P], in0=psd[:, 0:P],
                                    in1=mdiag[:], op=ALU.add)
                                if g > 0:
                                    nc.scalar.activation(
                                        out=ssb[:, P:width - P],
                                        in_=pss[:, 0:width - 2 * P],
                                        func=AF.Copy)
                                nc.vector.tensor_tensor(
                                    out=ssb[:, width - P:width],
                                    in0=pss[:, width - 2 * P:width - P],
                                    in1=mtris[:], op=ALU.add)
                            else:
                                pss = ps_a.tile([P, 512], f32, name="psq")
                                nc.tensor.matmul(
                                    out=pss[:, 0:width], lhsT=qst,
                                    rhs=kT[prow, mq, 512:512 + width],
                                    start=True, stop=True)
                                if width > P:
                                    nc.scalar.activation(
                                        out=ssb[:, 0:width - P],
                                        in_=pss[:, 0:width - P], func=AF.Copy)
                                nc.vector.tensor_tensor(
                                    out=ssb[:, width - P:width],
                                    in0=pss[:, width - P:width],
                                    in1=mtrii[:], op=ALU.add)
                            negmax = small.tile([P, 1], f32, name="negmax")
                            nc.vector.tensor_reduce(
                                out=negmax[:], in_=ssb[:, 0:width],
                                axis=AX.X, op=ALU.max, negate=True)
                            probs = sp.tile([P, 640], bf16, name="brow")
                            sums = small.tile([P, 1], f32, name="sums")
                            nc.scalar.activation(
                                out=probs[:, 0:width], in_=ssb[:, 0:width],
                                func=AF.Exp, bias=negmax[:],
                                accum_out=sums[:])
                            recip = small.tile([P, 1], f32, name="recip")
                            nc.vector.reciprocal(out=recip[:], in_=sums[:])
                            nc.vector.tensor_scalar(
                                out=probs[:, 0:width], in0=probs[:, 0:width],
                                scalar1=recip[:], scalar2=None, op0=ALU.mult)
                            if g < 4:
                                chunks = [(4, 0)] + [(mm, (mm + 1) * P)
                                                     for mm in range(g + 1)]
                            else:
                                chunks = [(mm, mm * P)
                                          for mm in range(g - 3)]
                            for ci, (slot, coff) in enumerate(chunks):
                                tpp = ps_tr.tile([P, P], bf16, name="trps")
                                nc.tensor.transpose(
                                    tpp[:], probs[:, coff:coff + P],
                                    ident[:])
                                if (g + ci) % 2 == 0:
                                    nc.scalar.activation(
                                        out=ptile[:, slot,
                                                  g * P:(g + 1) * P],
                                        in_=tpp[:], func=AF.Copy)
                                else:
                                    nc.vector.tensor_copy(
                                        out=ptile[:, slot,
                                                  g * P:(g + 1) * P],
                                        in_=tpp[:])
                        pav_lo = ps_av.tile([64, 512], f32, name="pav")
                        pav_hi = ps_av.tile([64, 512], f32, name="pav")
                        for mm in range(4):
                            nc.tensor.matmul(
                                out=pav_lo[:, mm * P:512],
                                lhsT=vv[:, 4 + mm, hl * HD:(hl + 1) * HD],
                                rhs=ptile[:, mm, mm * P:512],
                                start=(mm == 0), stop=False)
                            nc.tensor.matmul(
                                out=pav_hi[:, mm * P:512],
                                lhsT=vv[:, 4 + mm, hl * HD:(hl + 1) * HD],
                                rhs=ptile[:, mm, 512 + mm * P:T],
                                start=(mm == 0), stop=(mm == 3))
                        for g in range(4):
                            nc.tensor.matmul(
                                out=pav_lo[:, g * P:(g + 1) * P],
                                lhsT=vv[:, g, hl * HD:(hl + 1) * HD],
                                rhs=ptile[:, 4, g * P:(g + 1) * P],
                                start=False, stop=(g == 3))
                        if hl % 2 == 0:
                            nc.scalar.activation(out=oT[prow, mq, 0:512],
                                                 in_=pav_lo[:], func=AF.Copy)
                            nc.vector.tensor_copy(out=oT[prow, mq, 512:T],
                                                  in_=pav_hi[:])
                        else:
                            nc.vector.tensor_copy(out=oT[prow, mq, 0:512],
                                                  in_=pav_lo[:])
                            nc.scalar.activation(out=oT[prow, mq, 512:T],
                                                 in_=pav_hi[:], func=AF.Copy)
                    oproj_ar_update(oT, f"wos{l}", f"bos{l}")

                    # ======== cross attention ========
                    lns2, lnb2 = ln_consts(lnw_d[l, 1], lnb_d[l, 1])
                    x2T = pers.tile([P, ND, T], bf16, name="x2T", tag="xT")
                    ln_to_xT(x2T, lns2, lnb2)
                    q2T = pers.tile([P, MH, T], bf16, name="q2T", tag="qT")
                    bq2_sb = small.tile([P, MH], f32, name="bq")
                    nc.sync.dma_start(out=bq2_sb[:], in_=bd[f"bqc{l}"][:])
                    bk2_sb = small.tile([P, MH], f32, name="bk")
                    nc.sync.dma_start(out=bk2_sb[:], in_=bd[f"bkc{l}"][:])
                    bv2_b = lnp.tile([P, hsh * HD], bf16, name="bv_b")
                    nc.gpsimd.dma_start(out=bv2_b[:],
                                         in_=bcast(bd[f"bvc{l}"][0]))
                    proj_qk(q2T, f"wqc{l}", bq2_sb, 0.125, x2T)

                    wch_k = wp.tile([P, ND, MH, P], bf16, name="wch")
                    for m in range(MH):
                        nc.sync.dma_start(out=wch_k[:, :, m, :],
                                          in_=wd[f"wkc{l}"][m])
                    kenc_raw = sp.tile([S, hsh * HD], bf16, name="kenc", bufs=1)
                    psk = ps_a.tile([P, 512], f32, name="psq")
                    for k in range(ND):
                        nc.tensor.matmul(out=psk[:], lhsT=encT[:, k, :],
                                         rhs=wch_k[:, k, :, :],
                                         start=(k == 0), stop=(k == ND - 1))
                    nc.scalar.activation(out=kenc_raw[:], in_=psk[:],
                                         func=AF.Copy)
                    kTe = sp.tile([P, MH, S], bf16, name="kTe", bufs=1)
                    for m in range(MH):
                        tpk = ps_tr.tile([P, P], bf16, name="trps")
                        nc.tensor.transpose(
                            tpk[:], kenc_raw[:, m * P:(m + 1) * P], ident[:])
                        nc.vector.tensor_scalar(
                            out=kTe[:, m, :], in0=tpk[:],
                            scalar1=bk2_sb[:, m:m + 1], scalar2=None,
                            op0=ALU.add)
                    wch_v2 = wp.tile([P, ND, hsh * HD], bf16, name="wch")
                    nc.sync.dma_start(out=wch_v2[:], in_=wd[f"wvc{l}"][0])
                    venc = sp.tile([S, hsh * HD], bf16, name="venc", bufs=1)
                    psv2 = ps_a.tile([P, 512], f32, name="psq")
                    for k in range(ND):
                        nc.tensor.matmul(out=psv2[:], lhsT=encT[:, k, :],
                                         rhs=wch_v2[:, k, :],
                                         start=(k == 0), stop=(k == ND - 1))
                    nc.vector.tensor_tensor(out=venc[:], in0=psv2[:],
                                            in1=bv2_b[:], op=ALU.add)

                    o2T = pers.tile([P, KO, T], bf16, name="o2T", tag="oT")
                    for hl in range(hsh):
                        prow = slice((hl % 2) * 64, (hl % 2) * 64 + 64)
                        mq = hl // 2
                        p2tile = sp.tile([S, T], bf16, name="ptile")
                        for tt in range(NT):
                            ps2 = ps_a.tile([P, 512], f32, name="psq")
                            nc.tensor.matmul(
                                out=ps2[:, 0:S],
                                lhsT=q2T[prow, mq, tt * P:(tt + 1) * P],
                                rhs=kTe[prow, mq, :], start=True, stop=True)
                            s2 = sp.tile([P, S], f32, name="srow")
                            nc.vector.tensor_tensor(out=s2[:],
                                                    in0=ps2[:, 0:S],
                                                    in1=cmask_b[:],
                                                    op=ALU.add)
                            negmax = small.tile([P, 1], f32, name="negmax")
                            nc.vector.tensor_reduce(
                                out=negmax[:], in_=s2[:], axis=AX.X,
                                op=ALU.max, negate=True)
                            probs2 = sp.tile([P, S], bf16, name="brow")
                            sums = small.tile([P, 1], f32, name="sums")
                            nc.scalar.activation(
                                out=probs2[:], in_=s2[:], func=AF.Exp,
                                bias=negmax[:], accum_out=sums[:])
                            recip = small.tile([P, 1], f32, name="recip")
                            nc.vector.reciprocal(out=recip[:], in_=sums[:])
                            nc.vector.tensor_scalar(
                                out=probs2[:], in0=probs2[:],
                                scalar1=recip[:], scalar2=None, op0=ALU.mult)
                            tpp = ps_tr.tile([P, P], bf16, name="trps")
                            nc.tensor.transpose(tpp[:], probs2[:], ident[:])
                            nc.scalar.activation(
                                out=p2tile[:, tt * P:(tt + 1) * P],
                                in_=tpp[:], func=AF.Copy)
                        for half in range(2):
                            pav2 = ps_av.tile([64, 512], f32, name="pav")
                            nc.tensor.matmul(
                                out=pav2[:],
                                lhsT=venc[:, hl * HD:(hl + 1) * HD],
                                rhs=p2tile[:, half * 512:(half + 1) * 512],
                                start=True, stop=True)
                            nc.scalar.activation(
                                out=o2T[prow, mq, half * 512:(half + 1) * 512],
                                in_=pav2[:], func=AF.Copy)
                    oproj_ar_update(o2T, f"woc{l}", f"boc{l}")

                    # ======== FFN ========
                    NDT = dsh // P
                    lns3, lnb3 = ln_consts(lnw_d[l, 2], lnb_d[l, 2])
                    x3T = pers.tile([P, ND, T], bf16, name="x3T", tag="xT")
                    ln_to_xT(x3T, lns3, lnb3)
                    bf1_sb = small.tile([P, NDT], f32, name="bf1s")
                    nc.sync.dma_start(out=bf1_sb[:], in_=bd[f"bf1{l}"][:])
                    bf2_b = lnp.tile([P, D], bf16, name="bo_b")
                    nc.gpsimd.dma_start(out=bf2_b[:],
                                         in_=bcast(bd[f"bf2{l}"][0]))
                    # fc1: stationary weights -> g^T directly, fused
                    # bias+gelu on eviction
                    gT = pers.tile([P, NDT, T], bf16, name="gT", tag="oT")
                    for dfft in range(NDT):
                        f1c = wp.tile([P, ND, P], bf16, name="wch")
                        nc.sync.dma_start(out=f1c[:], in_=wd[f"wf1{l}"][dfft])
                        for th in range(2):
                            psf = ps_a.tile([P, 512], f32, name="psq")
                            for k in range(ND):
                                nc.tensor.matmul(
                                    out=psf[:], lhsT=f1c[:, k, :],
                                    rhs=x3T[:, k, th * 512:(th + 1) * 512],
                                    start=(k == 0), stop=(k == ND - 1))
                            nc.scalar.activation(
                                out=gT[:, dfft, th * 512:(th + 1) * 512],
                                in_=psf[:], func=gelu,
                                bias=bf1_sb[:, dfft:dfft + 1])
                    # fc2: accumulate over dff tiles in PSUM per (tt, n2)
                    f2cs = []
                    for n2 in range(2):
                        f2c = wp.tile([P, NDT, 512], bf16, name="wch")
                        nc.sync.dma_start(out=f2c[:], in_=wd[f"wf2{l}"][n2])
                        f2cs.append(f2c)
                    for rh in range(2):
                        for tl in range(4):
                            tt = 4 * rh + tl
                            for n2 in range(2):
                                psf2 = ps_a.tile([P, 512], f32, name="psq")
                                for kk in range(NDT):
                                    nc.tensor.matmul(
                                        out=psf2[:],
                                        lhsT=gT[:, kk, tt * P:(tt + 1) * P],
                                        rhs=f2cs[n2][:, kk, :],
                                        start=(kk == 0),
                                        stop=(kk == NDT - 1))
                                ev = xb.tile([P, 512], bf16, name="evb")
                                nc.vector.tensor_tensor(
                                    out=ev[:], in0=psf2[:],
                                    in1=bf2_b[:, n2 * 512:(n2 + 1) * 512],
                                    op=ALU.add)
                                nc.sync.dma_start(
                                    out=ccr_in[rh][tl * P:(tl + 1) * P,
                                                   n2 * 512:(n2 + 1) * 512],
                                    in_=ev[:])
                        if collectives:
                            nc.gpsimd.collective_compute(
                                "AllReduce", ALU.add, replica_groups=PAIRS,
                                ins=[ccr_in[rh][:]], outs=[ccr_out[rh][:]])
                        ar_h_update(rh)

                # ---- final LN -> ccf ----
                lnfs, lnfb = ln_consts(lnfin_d[0], lnfin_d[1])
                for rh in range(2):
                    for tl in range(4):
                        hf = xb.tile([P, D], bf16, name="xbrow")
                        ln_tile(h[:, 4 * rh + tl, :], hf[:], lnfs[:],
                                lnfb[:])
                        nc.sync.dma_start(
                            out=ccf_in[rh][tl * P:(tl + 1) * P, :],
                            in_=hf[:])
                    if collectives:
                        nc.gpsimd.collective_compute(
                            "AllGather", ALU.bypass, replica_groups=EVENODD,
                            ins=[ccf_in[rh][:]], outs=[ccf_out[rh][:]])
            # stack pools closed here

            # ---------------- LM head ----------------
            lctx = contextlib.ExitStack()
            with lctx:
                lmp = lctx.enter_context(tc.tile_pool(name="lmp", bufs=1))
                lmt = lctx.enter_context(tc.tile_pool(name="lmt", bufs=2))
                wlm_sb = lmp.tile([P, ND, vsh], bf16)
                for k in range(ND):
                    nc.sync.dma_start(out=wlm_sb[:, k, :], in_=wlm_d[:, k, :])
                for bt in range(nb_lm):
                    for tt in range(NT):
                        hft = xb.tile([P, D], bf16, name="xbrow")
                        rh, tl = tt // 4, tt % 4
                        src = (ccf_out[rh][bt, tl * P:(tl + 1) * P, :]
                               if collectives
                               else ccf_in[rh][tl * P:(tl + 1) * P, :])
                        nc.sync.dma_start(out=hft[:], in_=src)
                        hfT = lmt.tile([P, ND, P], bf16, name="hfT")
                        for k in range(ND):
                            tph = ps_tr.tile([P, P], bf16, name="trps")
                            nc.tensor.transpose(
                                tph[:], hft[:, k * P:(k + 1) * P], ident[:])
                            if k % 2 == 0:
                                nc.scalar.activation(out=hfT[:, k, :],
                                                     in_=tph[:],
                                                     func=AF.Copy)
                            else:
                                nc.vector.tensor_copy(out=hfT[:, k, :],
                                                      in_=tph[:])
                        for v in range(vsh // 512):
                            psl = ps_a.tile([P, 512], f32, name="psq")
                            for k in range(ND):
                                nc.tensor.matmul(
                                    out=psl[:], lhsT=hfT[:, k, :],
                                    rhs=wlm_sb[:, k, v * 512:(v + 1) * 512],
                                    start=(k == 0), stop=(k == ND - 1))
                            osb = xp.tile([P, 512], f32, name="ev512")
                            if v % 2 == 0:
                                nc.scalar.activation(out=osb[:], in_=psl[:],
                                                     func=AF.Copy)
                            else:
                                nc.vector.tensor_copy(out=osb[:], in_=psl[:])
                            nc.sync.dma_start(
                                out=out_d[(bt * NT + tt) * P:
                                          (bt * NT + tt + 1) * P,
                                          v * 512:(v + 1) * 512],
                                in_=osb[:])
    nc.compile()
    return nc


_NC_CACHE = {}


def _get_nc(key):
    if key not in _NC_CACHE:
        hsh, dsh, vsh, nb_lm, coll = key
        _NC_CACHE[key] = build_nc(hsh, dsh, vsh, nb_lm, coll)
    return _NC_CACHE[key]


def kernel(**inputs) -> np.ndarray:
    nc = _get_nc((8, 2048, VSH, 4, True))
    maps = host_prepare(inputs, hsh=8, dsh=2048, vsh=VSH)
    res = run_bass_kernel_spmd(nc, maps, core_ids=list(range(8)),
                               trace=False)
    logits = np.concatenate([res.results[c]["out"] for c in range(8)], axis=1)
    return np.ascontiguousarray(
        logits[:, : V + 1].reshape(B, T, V + 1).astype(np.float32))



# revision 10
# speedup vs baseline: 1.2084x; 1.2084x over previous
"""BlockDiffusionDecoder (mBART-style 2-layer decoder + BD3LM self-attn mask)
on 8 Trainium2 NeuronCores.

Sharding: sequence-parallel.  Core c owns batch b = c//2 and token half
h = c%2 (512 of the 1024 tokens).  Each core carries the FULL hidden state
in a uniform local layout: local tiles 0-3 = its own token half, local
tiles 4-7 = the global second half (x0 tokens).  For odd cores the two
regions coincide (own half duplicated), which keeps the SPMD program
identical across cores — only input data differs (ids/pos row order and
two 128x128 self-attention mask tiles).

Per layer each core computes K/V for all 8 local tiles but Q, attention,
cross-attention and FFN only for its own 4 tiles; sublayer deltas are
added straight into the fp32 residual (no collectives).  After each
non-final layer a single pair AllGather (1 MB bf16) refreshes local
tiles 4-7.  The LM head is token-parallel: full-vocab weights are
streamed from HBM and each core emits logits for its own 512 tokens
(bf16), so no final AllGather is needed.

LayerNorm scale/bias are folded into the downstream projection weights /
biases host-side, so on-device LN is just (x - mu) * rsqrt(var + eps).
"""
import sys

if "/opt/trn_rl_repo" not in sys.path:
    sys.path.insert(0, "/opt/trn_rl_repo")

import contextlib

import ml_dtypes
import numpy as np

import concourse.bass as bass
import concourse.bacc as bacc
import concourse.tile as tile
from concourse import mybir
from concourse.bass_utils import run_bass_kernel_spmd
from concourse.masks import make_identity

P = 128
B, D, H, NL, DFF, V, S = 4, 1024, 16, 2, 4096, 32000, 128
T = 1024
T2 = 512             # tokens owned per core
HD = D // H          # 64
BLK = 4
VP = 32768           # padded vocab
NT = 8               # local token tiles (4 own + 4 "x0 region")
NTO = 4              # own token tiles
ND = D // P          # 8 feature tiles
MH = ND              # full heads per core -> 8 m-tiles for q/k
NVC = VP // 512      # 64 lm-head vocab chunks
EMB_SCALE = 32.0     # sqrt(D)
FMIN = float(np.finfo(np.float32).min)
BF = ml_dtypes.bfloat16

f32 = mybir.dt.float32
bf16 = mybir.dt.bfloat16
i32 = mybir.dt.int32
AF = mybir.ActivationFunctionType
ALU = mybir.AluOpType
AX = mybir.AxisListType

PAIRS = [[0, 1], [2, 3], [4, 5], [6, 7]]


def _rhs_tile(w_t: np.ndarray, nchunk: int) -> np.ndarray:
    """[d_in, d_out] -> [n_chunks, 128, k_tiles, nchunk] bf16."""
    d_in, d_out = w_t.shape
    kt = d_in // P
    nc_ = d_out // nchunk
    return np.ascontiguousarray(
        w_t.reshape(kt, P, nc_, nchunk).transpose(2, 1, 0, 3).astype(BF))


def _mask_consts():
    i = np.arange(P)
    diag = np.where((i[:, None] // BLK) == (i[None, :] // BLK), 0.0, FMIN)
    tri_s = np.where((i[:, None] // BLK) > (i[None, :] // BLK), 0.0, FMIN)
    tri_i = np.where((i[:, None] // BLK) >= (i[None, :] // BLK), 0.0, FMIN)
    return (diag.astype(np.float32), tri_s.astype(np.float32),
            tri_i.astype(np.float32))


def host_prepare(inputs: dict):
    ids = np.asarray(inputs["input_ids"])
    enc = np.asarray(inputs["enc_hidden"], dtype=np.float32)
    emask = np.asarray(inputs["enc_mask"])
    emb = np.ascontiguousarray(np.asarray(inputs["embed_tokens"], np.float32))
    pos = np.asarray(inputs["pos_embed"], np.float32)
    attn_w = np.asarray(inputs["attn_w"], np.float32)
    attn_b = np.asarray(inputs["attn_b"], np.float32)
    ln_w = np.asarray(inputs["ln_w"], np.float32)
    ln_b = np.asarray(inputs["ln_b"], np.float32)
    fc1_w = np.asarray(inputs["fc1_w"], np.float32)
    fc1_b = np.asarray(inputs["fc1_b"], np.float32)
    fc2_w = np.asarray(inputs["fc2_w"], np.float32)
    fc2_b = np.asarray(inputs["fc2_b"], np.float32)
    lm_w = np.asarray(inputs["lm_head_w"], np.float32)
    fs = np.asarray(inputs["final_ln_s"], np.float32)
    fb = np.asarray(inputs["final_ln_b"], np.float32)

    # final LN fold into lm head
    lm_pad = np.zeros((VP, D), np.float32)
    lm_pad[: V + 1] = lm_w
    lm_t = lm_pad.T * fs[:, None]                      # [D, VP]
    logit_bias = fb @ lm_pad.T                         # [VP] host-added
    wlm = np.ascontiguousarray(
        lm_t.reshape(ND, P, NVC, 512).transpose(2, 1, 0, 3).astype(BF))

    mdiag, mtris, mtrii = _mask_consts()
    allneg = np.full((P, P), FMIN, np.float32)

    # shared (core-independent) weight tensors
    shared = {"emb": emb, "wlm": wlm,
              "lnemb": np.stack([np.asarray(inputs["ln_emb_s"], np.float32),
                                 np.asarray(inputs["ln_emb_b"], np.float32)])}
    for l in range(NL):
        for a, tag in ((0, "s"), (1, "c")):
            wq, wk, wv, wo = attn_w[l, a]
            bq, bk, bv, bo = attn_b[l, a]
            s_ln = ln_w[l, a] if a < 2 else None
            b_ln = ln_b[l, a]
            s_ln = ln_w[l, a]
            # fold LN affine into x-side projections (q always; k,v only
            # for self-attn where they read the LN'd x)
            wqT = wq.T * s_ln[:, None]
            bq_e = bq + b_ln @ wq.T
            if a == 0:
                wkT = wk.T * s_ln[:, None]
                bk_e = bk + b_ln @ wk.T
                wvT = wv.T * s_ln[:, None]
                bv_e = bv + b_ln @ wv.T
            else:
                wkT, bk_e, wvT, bv_e = wk.T, bk, wv.T, bv
            shared[f"wq{tag}{l}"] = _rhs_tile(wqT, P)
            shared[f"wk{tag}{l}"] = _rhs_tile(wkT, P)
            shared[f"wv{tag}{l}"] = _rhs_tile(wvT, 512)
            shared[f"wo{tag}{l}"] = _rhs_tile(wo.T, 512)
            shared[f"bq{tag}{l}"] = np.ascontiguousarray(
                bq_e.reshape(MH, P).T)
            shared[f"bk{tag}{l}"] = np.ascontiguousarray(
                bk_e.reshape(MH, P).T)
            shared[f"bv{tag}{l}"] = bv_e.reshape(1, D).copy()
            shared[f"bo{tag}{l}"] = bo.reshape(1, D).copy()
        s3, b3 = ln_w[l, 2], ln_b[l, 2]
        f1T = fc1_w[l].T * s3[:, None]
        bf1_e = fc1_b[l] + b3 @ fc1_w[l].T
        shared[f"wf1{l}"] = _rhs_tile(f1T, P)
        shared[f"bf1{l}"] = np.ascontiguousarray(
            bf1_e.reshape(DFF // P, P).T)
        # fc2 streamed as [kk-group of 8][n2] chunks: [4, 2, P, 8, 512]
        shared[f"wf2{l}"] = np.ascontiguousarray(
            fc2_w[l].T.reshape(4, 8, P, 2, 512)
            .transpose(0, 3, 2, 1, 4).astype(BF))
        shared[f"bf2{l}"] = fc2_b[l].reshape(1, D).copy()

    maps = []
    for c in range(8):
        b_, half = c // 2, c % 2
        own = slice(half * T2, half * T2 + T2)
        x0 = slice(T2, T)
        m = dict(shared)
        m["ids"] = np.concatenate(
            [ids[b_, own], ids[b_, x0]]).reshape(T, 1).astype(np.int32)
        m["pos"] = np.ascontiguousarray(
            np.concatenate([pos[own], pos[x0]], axis=0))
        m["encT"] = np.ascontiguousarray(enc[b_].T.astype(BF))
        m["cmask"] = ((1.0 - emask[b_].astype(np.float32)) * FMIN
                      ).reshape(1, S)
        m["mA"] = mdiag if half == 0 else mtrii
        m["mB"] = mtris if half == 0 else allneg
        maps.append(m)
    return maps, logit_bias


def build_nc(collectives=True, gelu=AF.Gelu_apprx_tanh):
    nc = bacc.Bacc(num_devices=8 if collectives else None, trn_type="TRN2")

    ids_d = nc.dram_tensor("ids", [T, 1], i32, kind="ExternalInput")
    emb_d = nc.dram_tensor("emb", [V + 1, D], f32, kind="ExternalInput")
    pos_d = nc.dram_tensor("pos", [T, D], f32, kind="ExternalInput")
    encT_d = nc.dram_tensor("encT", [D, S], bf16, kind="ExternalInput")
    cmask_d = nc.dram_tensor("cmask", [1, S], f32, kind="ExternalInput")
    lnemb_d = nc.dram_tensor("lnemb", [2, D], f32, kind="ExternalInput")
    mA_d = nc.dram_tensor("mA", [P, P], f32, kind="ExternalInput")
    mB_d = nc.dram_tensor("mB", [P, P], f32, kind="ExternalInput")
    wlm_d = nc.dram_tensor("wlm", [NVC, P, ND, 512], bf16,
                           kind="ExternalInput")
    wd, bd = {}, {}
    for l in range(NL):
        for tg in ("s", "c"):
            wd[f"wq{tg}{l}"] = nc.dram_tensor(
                f"wq{tg}{l}", [MH, P, ND, P], bf16, kind="ExternalInput")
            wd[f"wk{tg}{l}"] = nc.dram_tensor(
                f"wk{tg}{l}", [MH, P, ND, P], bf16, kind="ExternalInput")
            wd[f"wv{tg}{l}"] = nc.dram_tensor(
                f"wv{tg}{l}", [2, P, ND, 512], bf16, kind="ExternalInput")
            wd[f"wo{tg}{l}"] = nc.dram_tensor(
                f"wo{tg}{l}", [2, P, ND, 512], bf16, kind="ExternalInput")
            bd[f"bq{tg}{l}"] = nc.dram_tensor(
                f"bq{tg}{l}", [P, MH], f32, kind="ExternalInput")
            bd[f"bk{tg}{l}"] = nc.dram_tensor(
                f"bk{tg}{l}", [P, MH], f32, kind="ExternalInput")
            bd[f"bv{tg}{l}"] = nc.dram_tensor(
                f"bv{tg}{l}", [1, D], f32, kind="ExternalInput")
            bd[f"bo{tg}{l}"] = nc.dram_tensor(
                f"bo{tg}{l}", [1, D], f32, kind="ExternalInput")
        wd[f"wf1{l}"] = nc.dram_tensor(
            f"wf1{l}", [DFF // P, P, ND, P], bf16, kind="ExternalInput")
        bd[f"bf1{l}"] = nc.dram_tensor(
            f"bf1{l}", [P, DFF // P], f32, kind="ExternalInput")
        wd[f"wf2{l}"] = nc.dram_tensor(
            f"wf2{l}", [4, 2, P, 8, 512], bf16, kind="ExternalInput")
        bd[f"bf2{l}"] = nc.dram_tensor(
            f"bf2{l}", [1, D], f32, kind="ExternalInput")
    out_d = nc.dram_tensor("out", [T2, VP], bf16, kind="ExternalOutput")

    def bcast(ap_1d, p=P):
        return bass.AP(tensor=ap_1d.tensor, offset=ap_1d.offset,
                       ap=[[0, p]] + list(ap_1d.ap))

    with tile.TileContext(nc) as tc:
        gctx = contextlib.ExitStack()
        with gctx:
            consts = gctx.enter_context(tc.tile_pool(name="consts", bufs=1))
            small = gctx.enter_context(tc.tile_pool(name="small", bufs=4))
            sp = gctx.enter_context(tc.tile_pool(name="sp", bufs=2))
            xb = gctx.enter_context(tc.tile_pool(name="xb", bufs=2))
            dram = gctx.enter_context(
                tc.tile_pool(name="dram", bufs=1, space="DRAM"))
            ps_a = gctx.enter_context(
                tc.tile_pool(name="ps_a", bufs=4, space="PSUM"))
            ps_av = gctx.enter_context(
                tc.tile_pool(name="ps_av", bufs=2, space="PSUM"))
            ps_tr = gctx.enter_context(
                tc.tile_pool(name="ps_tr", bufs=2, space="PSUM"))
            glob = gctx.enter_context(tc.tile_pool(name="glob", bufs=1))

            ident = consts.tile([P, P], bf16)
            make_identity(nc, ident[:])
            eps_t = consts.tile([P, 1], f32)
            nc.vector.memset(eps_t[:], 1e-5)
            mA = consts.tile([P, P], f32)
            nc.sync.dma_start(out=mA[:], in_=mA_d[:])
            mB = consts.tile([P, P], f32)
            nc.sync.dma_start(out=mB[:], in_=mB_d[:])
            cmask_b = consts.tile([P, S], f32)
            nc.sync.dma_start(out=cmask_b[:], in_=bcast(cmask_d[0]))

            # final hidden (transposed) for the LM head — outlives the
            # stack pools
            hT = glob.tile([P, ND, T2], bf16)

            ccx_in = dram.tile([T2, D], bf16, name="ccx_in")
            ccx_out = (dram.tile([2, T2, D], bf16, name="ccx_out")
                       if collectives else None)

            def ln_stats(src_ap, par):
                """-> (negmur [P,1], rstd [P,1]): x_hat = x*rstd + negmur."""
                st = small.tile([P, 2, 6], f32, name="lnstats")
                nc.vector.bn_stats(out=st[:, 0, :], in_=src_ap[:, 0:512])
                nc.vector.bn_stats(out=st[:, 1, :], in_=src_ap[:, 512:1024])
                mv = small.tile([P, 2], f32, name="lnmv")
                nc.vector.bn_aggr(out=mv[:], in_=st[:])
                rstd = small.tile([P, 1], f32, name="lnrstd")
                nc.scalar.activation(out=rstd[:], in_=mv[:, 1:2],
                                     func=AF.Sqrt, bias=eps_t[:])
                nc.vector.reciprocal(out=rstd[:], in_=rstd[:])
                negmur = small.tile([P, 1], f32, name="lnnm")
                nc.vector.tensor_scalar(out=negmur[:], in0=mv[:, 0:1],
                                        scalar1=rstd[:], scalar2=-1.0,
                                        op0=ALU.mult, op1=ALU.mult)
                return negmur, rstd

            def ln_pure(src_ap, dst_ap, par):
                """dst = (src - mean) * rsqrt(var+eps); engine by parity."""
                negmur, rstd = ln_stats(src_ap, par)
                if par % 2 == 0:
                    nc.scalar.activation(out=dst_ap, in_=src_ap,
                                         func=AF.Identity,
                                         scale=rstd[:], bias=negmur[:])
                else:
                    nc.vector.tensor_scalar(out=dst_ap, in0=src_ap,
                                            scalar1=rstd[:],
                                            scalar2=negmur[:],
                                            op0=ALU.mult, op1=ALU.add)
                return negmur, rstd

            # ---------------- stack phase ----------------
            sctx = contextlib.ExitStack()
            with sctx:
                pers = sctx.enter_context(tc.tile_pool(name="pers", bufs=1))
                wp = sctx.enter_context(tc.tile_pool(name="wp", bufs=2))
                lnp = sctx.enter_context(tc.tile_pool(name="lnp", bufs=1))

                h = pers.tile([P, NT, D], f32)
                encT = pers.tile([P, ND, S], bf16)
                nc.sync.dma_start(
                    out=encT[:],
                    in_=encT_d.rearrange("(k p) s -> p k s", p=P))
                # cross-attn K/V for both layers, precomputed
                encKT = pers.tile([P, NL, MH, S], bf16)
                encV = pers.tile([P, NL, D], bf16)

                def bias_bcast(src_row, name):
                    t = lnp.tile([P, D], bf16, name=name, tag="bb", bufs=2)
                    nc.gpsimd.dma_start(out=t[:], in_=bcast(src_row))
                    return t

                def transpose_to(dst_ap, src_ap, par):
                    tp_ = ps_tr.tile([P, P], bf16, name="trps")
                    nc.tensor.transpose(tp_[:], src_ap, ident[:])
                    if par % 2 == 0:
                        nc.scalar.activation(out=dst_ap, in_=tp_[:],
                                             func=AF.Copy)
                    else:
                        nc.vector.tensor_copy(out=dst_ap, in_=tp_[:])

                # ---- embed + emb LN (general affine) ----
                with nc.named_scope("embed"):
                    lnes = bias_bcast(lnemb_d[0], "lnes")
                    lneb = bias_bcast(lnemb_d[1], "lneb")
                    for tt in range(NT):
                        idt = small.tile([P, 1], i32, name="idt")
                        nc.sync.dma_start(out=idt[:],
                                          in_=ids_d[tt * P:(tt + 1) * P])
                        g = xb.tile([P, D], f32, name="xrow", tag="xf",
                                    bufs=4)
                        nc.gpsimd.indirect_dma_start(
                            out=g[:], out_offset=None, in_=emb_d[:],
                            in_offset=bass.IndirectOffsetOnAxis(
                                ap=idt[:, :1], axis=0))
                        pt = xb.tile([P, D], f32, name="xrow2", tag="xf",
                                     bufs=4)
                        nc.sync.dma_start(out=pt[:],
                                          in_=pos_d[tt * P:(tt + 1) * P])
                        nc.vector.tensor_scalar(out=g[:], in0=g[:],
                                                scalar1=EMB_SCALE,
                                                scalar2=None, op0=ALU.mult)
                        nc.vector.tensor_tensor(out=g[:], in0=g[:],
                                                in1=pt[:], op=ALU.add)
                        negmur, rstd = ln_stats(g[:], tt)
                        nc.scalar.activation(out=pt[:], in_=g[:],
                                             func=AF.Identity,
                                             scale=rstd[:],
                                             bias=negmur[:])
                        nc.vector.tensor_tensor(out=pt[:], in0=pt[:],
                                                in1=lnes[:], op=ALU.mult)
                        nc.vector.tensor_tensor(out=h[:, tt, :], in0=pt[:],
                                                in1=lneb[:], op=ALU.add)

                    # enc K/V for both layers (independent of h)
                    for l in range(NL):
                        bk2 = small.tile([P, MH], f32, name="bk2")
                        nc.sync.dma_start(out=bk2[:], in_=bd[f"bkc{l}"][:])
                        bv2 = bias_bcast(bd[f"bvc{l}"][0], "bv2")
                        wck = wp.tile([P, ND, MH, P], bf16, name="wcc",
                                      tag="wcc", bufs=1)
                        for mq in range(MH):
                            nc.sync.dma_start(out=wck[:, :, mq, :],
                                              in_=wd[f"wkc{l}"][mq])
                        kraw = sp.tile([S, D], bf16, name="kraw", bufs=1)
                        for hf in range(2):
                            psk = ps_a.tile([P, 512], f32, name="psq")
                            for k in range(ND):
                                nc.tensor.matmul(
                                    out=psk[:], lhsT=encT[:, k, :],
                                    rhs=wck[:, k, 4 * hf:4 * hf + 4, :],
                                    start=(k == 0), stop=(k == ND - 1))
                            nc.scalar.activation(
                                out=kraw[:, hf * 512:(hf + 1) * 512],
                                in_=psk[:], func=AF.Copy)
                        for mq in range(MH):
                            tpk = ps_tr.tile([P, P], bf16, name="trps")
                            nc.tensor.transpose(
                                tpk[:], kraw[:, mq * P:(mq + 1) * P],
                                ident[:])
                            nc.vector.tensor_scalar(
                                out=encKT[:, l, mq, :], in0=tpk[:],
                                scalar1=bk2[:, mq:mq + 1], scalar2=None,
                                op0=ALU.add)
                        for hf in range(2):
                            wcv = wp.tile([P, ND, 512], bf16, name="wch",
                                          tag="wch")
                            nc.sync.dma_start(out=wcv[:],
                                              in_=wd[f"wvc{l}"][hf])
                            psv = ps_a.tile([P, 512], f32, name="psq")
                            for k in range(ND):
                                nc.tensor.matmul(
                                    out=psv[:], lhsT=encT[:, k, :],
                                    rhs=wcv[:, k, :],
                                    start=(k == 0), stop=(k == ND - 1))
                            nc.vector.tensor_tensor(
                                out=encV[:, l, hf * 512:(hf + 1) * 512],
                                in0=psv[:],
                                in1=bv2[:, hf * 512:(hf + 1) * 512],
                                op=ALU.add)

                def proj_qk(dst, w_key, b_sb, scale, src_xT, ncols):
                    """dst[:, m, 0:ncols] = (x @ W)^T + b, x = src_xT cols."""
                    for m in range(MH):
                        wch = wp.tile([P, ND, P], bf16, name="wch", tag="wch")
                        nc.sync.dma_start(out=wch[:], in_=wd[w_key][m])
                        for cb in range(ncols // 512):
                            psq = ps_a.tile([P, 512], f32, name="psq")
                            for k in range(ND):
                                nc.tensor.matmul(
                                    out=psq[:], lhsT=wch[:, k, :],
                                    rhs=src_xT[:, k,
                                               cb * 512:(cb + 1) * 512],
                                    start=(k == 0), stop=(k == ND - 1))
                            if scale is None:
                                nc.vector.tensor_scalar(
                                    out=dst[:, m, cb * 512:(cb + 1) * 512],
                                    in0=psq[:], scalar1=b_sb[:, m:m + 1],
                                    scalar2=None, op0=ALU.add)
                            else:
                                nc.vector.tensor_scalar(
                                    out=dst[:, m, cb * 512:(cb + 1) * 512],
                                    in0=psq[:], scalar1=b_sb[:, m:m + 1],
                                    scalar2=scale, op0=ALU.add, op1=ALU.mult)

                def oproj_update(src_oT, wo_key, bo_key):
                    """h[own] += oT @ Wo + bo (bias via pre-add)."""
                    bo_b = bias_bcast(bd[bo_key][0], "bo_b")
                    for tt in range(NTO):
                        nc.vector.tensor_tensor(out=h[:, tt, :],
                                                in0=h[:, tt, :],
                                                in1=bo_b[:], op=ALU.add)
                    for n2 in range(2):
                        wch = wp.tile([P, ND, 512], bf16, name="wch",
                                      tag="wch")
                        nc.sync.dma_start(out=wch[:], in_=wd[wo_key][n2])
                        for tt in range(NTO):
                            pso = ps_a.tile([P, 512], f32, name="psq")
                            for k in range(ND):
                                nc.tensor.matmul(
                                    out=pso[:],
                                    lhsT=src_oT[:, k, tt * P:(tt + 1) * P],
                                    rhs=wch[:, k, :],
                                    start=(k == 0), stop=(k == ND - 1))
                            nc.vector.tensor_tensor(
                                out=h[:, tt, n2 * 512:(n2 + 1) * 512],
                                in0=h[:, tt, n2 * 512:(n2 + 1) * 512],
                                in1=pso[:], op=ALU.add)

                for l in range(NL):
                  with nc.named_scope(f"layer{l}"):
                    # ======== self attention ========
                    xT = pers.tile([P, ND, T], bf16, name="xT", tag="xT")
                    for tt in range(NT):
                        xt_ = xb.tile([P, D], bf16, name="xbrow", tag="xh")
                        ln_pure(h[:, tt, :], xt_[:], tt)
                        for k in range(ND):
                            transpose_to(xT[:, k, tt * P:(tt + 1) * P],
                                         xt_[:, k * P:(k + 1) * P], k)

                    qT = pers.tile([P, MH, T2], bf16, name="qT", tag="qT")
                    kT = pers.tile([P, MH, T], bf16, name="kT", tag="kT")
                    vv = pers.tile([P, NT, D], bf16, name="vv", tag="vv")
                    bq_sb = small.tile([P, MH], f32, name="bq")
                    nc.sync.dma_start(out=bq_sb[:], in_=bd[f"bqs{l}"][:])
                    bk_sb = small.tile([P, MH], f32, name="bk")
                    nc.sync.dma_start(out=bk_sb[:], in_=bd[f"bks{l}"][:])
                    bv_b = bias_bcast(bd[f"bvs{l}"][0], "bv_b")

                    proj_qk(qT, f"wqs{l}", bq_sb, 0.125, xT, T2)
                    proj_qk(kT, f"wks{l}", bk_sb, None, xT, T)
                    for hf in range(2):
                        wch_v = wp.tile([P, ND, 512], bf16, name="wch",
                                        tag="wch")
                        nc.sync.dma_start(out=wch_v[:], in_=wd[f"wvs{l}"][hf])
                        for tt in range(NT):
                            psv = ps_a.tile([P, 512], f32, name="psq")
                            for k in range(ND):
                                nc.tensor.matmul(
                                    out=psv[:],
                                    lhsT=xT[:, k, tt * P:(tt + 1) * P],
                                    rhs=wch_v[:, k, :],
                                    start=(k == 0), stop=(k == ND - 1))
                            nc.vector.tensor_tensor(
                                out=vv[:, tt, hf * 512:(hf + 1) * 512],
                                in0=psv[:],
                                in1=bv_b[:, hf * 512:(hf + 1) * 512],
                                op=ALU.add)

                    # scores + softmax + AV per head (own 4 query tiles)
                    oT = pers.tile([P, ND, T2], bf16, name="oT", tag="oT")
                    for hl in range(H):
                        prow = slice((hl % 2) * 64, (hl % 2) * 64 + 64)
                        mq = hl // 2
                        hds = slice(hl * HD, (hl + 1) * HD)
                        ptile = sp.tile([P, 5, T2], bf16, name="ptile")
                        for qi in range(NTO):
                            width = (qi + 2) * P
                            qst = qT[prow, mq, qi * P:(qi + 1) * P]
                            ssb = sp.tile([P, 640], f32, name="srow")
                            psd = ps_a.tile([P, 512], f32, name="psq")
                            nc.tensor.matmul(
                                out=psd[:, 0:P], lhsT=qst,
                                rhs=kT[prow, mq, qi * P:(qi + 1) * P],
                                start=True, stop=True)
                            pss = ps_a.tile([P, 512], f32, name="psq")
                            nc.tensor.matmul(
                                out=pss[:, 0:width - P], lhsT=qst,
                                rhs=kT[prow, mq, 512:512 + width - P],
                                start=True, stop=True)
                            nc.vector.tensor_tensor(
                                out=ssb[:, 0:P], in0=psd[:, 0:P],
                                in1=mA[:], op=ALU.add)
                            if qi > 0:
                                nc.scalar.activation(
                                    out=ssb[:, P:width - P],
                                    in_=pss[:, 0:width - 2 * P],
                                    func=AF.Copy)
                            nc.vector.tensor_tensor(
                                out=ssb[:, width - P:width],
                                in0=pss[:, width - 2 * P:width - P],
                                in1=mB[:], op=ALU.add)
                            negmax = small.tile([P, 1], f32, name="negmax")
                            nc.vector.tensor_reduce(
                                out=negmax[:], in_=ssb[:, 0:width],
                                axis=AX.X, op=ALU.max, negate=True)
                            probs = sp.tile([P, 640], bf16, name="brow")
                            sums = small.tile([P, 1], f32, name="sums")
                            nc.scalar.activation(
                                out=probs[:, 0:width], in_=ssb[:, 0:width],
                                func=AF.Exp, bias=negmax[:],
                                accum_out=sums[:])
                            recip = small.tile([P, 1], f32, name="recip")
                            nc.vector.reciprocal(out=recip[:], in_=sums[:])
                            nc.vector.tensor_scalar(
                                out=probs[:, 0:width], in0=probs[:, 0:width],
                                scalar1=recip[:], scalar2=None, op0=ALU.mult)
                            transpose_to(ptile[:, 4, qi * P:(qi + 1) * P],
                                         probs[:, 0:P], qi)
                            for j in range(qi + 1):
                                transpose_to(
                                    ptile[:, j, qi * P:(qi + 1) * P],
                                    probs[:, (j + 1) * P:(j + 2) * P],
                                    qi + j + 1)
                        pav = ps_av.tile([64, T2], f32, name="pav")
                        for s_ in range(4):
                            nc.tensor.matmul(
                                out=pav[:, s_ * P:T2],
                                lhsT=vv[:, 4 + s_, hds],
                                rhs=ptile[:, s_, s_ * P:T2],
                                start=(s_ == 0), stop=False)
                        for qi in range(NTO):
                            nc.tensor.matmul(
                                out=pav[:, qi * P:(qi + 1) * P],
                                lhsT=vv[:, qi, hds],
                                rhs=ptile[:, 4, qi * P:(qi + 1) * P],
                                start=False, stop=(qi == 3))
                        if hl % 2 == 0:
                            nc.scalar.activation(out=oT[prow, mq, :],
                                                 in_=pav[:], func=AF.Copy)
                        else:
                            nc.vector.tensor_copy(out=oT[prow, mq, :],
                                                  in_=pav[:])
                    oproj_update(oT, f"wos{l}", f"bos{l}")

                    # ======== cross attention ========
                    x2T = pers.tile([P, ND, T2], bf16, name="x2T", tag="x2T")
                    for ti in range(NTO):
                        xt_ = xb.tile([P, D], bf16, name="xbrow", tag="xh")
                        ln_pure(h[:, ti, :], xt_[:], ti)
                        for k in range(ND):
                            transpose_to(x2T[:, k, ti * P:(ti + 1) * P],
                                         xt_[:, k * P:(k + 1) * P], k)
                    q2T = pers.tile([P, MH, T2], bf16, name="q2T", tag="qT")
                    bq2_sb = small.tile([P, MH], f32, name="bq2")
                    nc.sync.dma_start(out=bq2_sb[:], in_=bd[f"bqc{l}"][:])
                    proj_qk(q2T, f"wqc{l}", bq2_sb, 0.125, x2T, T2)

                    o2T = pers.tile([P, ND, T2], bf16, name="o2T", tag="oT")
                    for hl in range(H):
                        prow = slice((hl % 2) * 64, (hl % 2) * 64 + 64)
                        mq = hl // 2
                        hds = slice(hl * HD, (hl + 1) * HD)
                        p2tile = sp.tile([S, T2], bf16, name="ptile",
                                         tag="ptile")
                        for qi in range(NTO):
                            ps2 = ps_a.tile([P, 512], f32, name="psq")
                            nc.tensor.matmul(
                                out=ps2[:, 0:S],
                                lhsT=q2T[prow, mq, qi * P:(qi + 1) * P],
                                rhs=encKT[prow, l, mq, :],
                                start=True, stop=True)
                            s2 = sp.tile([P, S], f32, name="srow",
                                         tag="srow")
                            nc.vector.tensor_tensor(out=s2[:],
                                                    in0=ps2[:, 0:S],
                                                    in1=cmask_b[:],
                                                    op=ALU.add)
                            negmax = small.tile([P, 1], f32, name="negmax")
                            nc.vector.tensor_reduce(
                                out=negmax[:], in_=s2[:], axis=AX.X,
                                op=ALU.max, negate=True)
                            probs2 = sp.tile([P, S], bf16, name="brow",
                                             tag="brow")
                            sums = small.tile([P, 1], f32, name="sums")
                            nc.scalar.activation(
                                out=probs2[:], in_=s2[:], func=AF.Exp,
                                bias=negmax[:], accum_out=sums[:])
                            recip = small.tile([P, 1], f32, name="recip")
                            nc.vector.reciprocal(out=recip[:], in_=sums[:])
                            nc.vector.tensor_scalar(
                                out=probs2[:], in0=probs2[:],
                                scalar1=recip[:], scalar2=None, op0=ALU.mult)
                            transpose_to(p2tile[:, qi * P:(qi + 1) * P],
                                         probs2[:], qi)
                        pav2 = ps_av.tile([64, T2], f32, name="pav")
                        nc.tensor.matmul(out=pav2[:], lhsT=encV[:, l, hds],
                                         rhs=p2tile[:], start=True,
                                         stop=True)
                        if hl % 2 == 0:
                            nc.scalar.activation(out=o2T[prow, mq, :],
                                                 in_=pav2[:], func=AF.Copy)
                        else:
                            nc.vector.tensor_copy(out=o2T[prow, mq, :],
                                                  in_=pav2[:])
                    oproj_update(o2T, f"woc{l}", f"boc{l}")

                    # ======== FFN ========
                    x3T = pers.tile([P, ND, T2], bf16, name="x3T", tag="x2T")
                    for ti in range(NTO):
                        xt_ = xb.tile([P, D], bf16, name="xbrow", tag="xh")
                        ln_pure(h[:, ti, :], xt_[:], ti)
                        for k in range(ND):
                            transpose_to(x3T[:, k, ti * P:(ti + 1) * P],
                                         xt_[:, k * P:(k + 1) * P], k)
                    bf1_sb = small.tile([P, DFF // P], f32, name="bf1s")
                    nc.sync.dma_start(out=bf1_sb[:], in_=bd[f"bf1{l}"][:])
                    bf2_b = bias_bcast(bd[f"bf2{l}"][0], "bf2_b")
                    for tt in range(NTO):
                        nc.vector.tensor_tensor(out=h[:, tt, :],
                                                in0=h[:, tt, :],
                                                in1=bf2_b[:], op=ALU.add)
                    for dh in range(2):
                        gT = pers.tile([P, 16, T2], bf16, name="gT",
                                       tag="xT")
                        for df in range(16):
                            dff = dh * 16 + df
                            f1c = wp.tile([P, ND, P], bf16, name="wch",
                                          tag="wch")
                            nc.sync.dma_start(out=f1c[:],
                                              in_=wd[f"wf1{l}"][dff])
                            psf = ps_a.tile([P, 512], f32, name="psq")
                            for k in range(ND):
                                nc.tensor.matmul(
                                    out=psf[:], lhsT=f1c[:, k, :],
                                    rhs=x3T[:, k, :],
                                    start=(k == 0), stop=(k == ND - 1))
                            nc.scalar.activation(
                                out=gT[:, df, :], in_=psf[:], func=gelu,
                                bias=bf1_sb[:, dff:dff + 1])
                        for n2 in range(2):
                            psf2s = [ps_a.tile([P, 512], f32, name="psq")
                                     for _ in range(NTO)]
                            for kg in range(2):
                                f2c = wp.tile([P, 8, 512], bf16, name="wch",
                                              tag="wch")
                                nc.sync.dma_start(
                                    out=f2c[:],
                                    in_=wd[f"wf2{l}"][dh * 2 + kg, n2])
                                for tt in range(NTO):
                                    for kk in range(8):
                                        nc.tensor.matmul(
                                            out=psf2s[tt][:],
                                            lhsT=gT[:, kg * 8 + kk,
                                                    tt * P:(tt + 1) * P],
                                            rhs=f2c[:, kk, :],
                                            start=(kg == 0 and kk == 0),
                                            stop=(kg == 1 and kk == 7))
                            for tt in range(NTO):
                                nc.vector.tensor_tensor(
                                    out=h[:, tt, n2 * 512:(n2 + 1) * 512],
                                    in0=h[:, tt, n2 * 512:(n2 + 1) * 512],
                                    in1=psf2s[tt][:], op=ALU.add)

                    # ======== pair exchange (not after last layer) ========
                    if l < NL - 1 and collectives:
                        for tl in range(NTO):
                            nc.gpsimd.dma_start(
                                out=ccx_in[tl * P:(tl + 1) * P, :],
                                in_=h[:, tl, :])
                        nc.gpsimd.collective_compute(
                            "AllGather", ALU.bypass, replica_groups=PAIRS,
                            ins=[ccx_in[:]], outs=[ccx_out[:]])
                        for tl in range(NTO):
                            nc.gpsimd.dma_start(
                                out=h[:, 4 + tl, :],
                                in_=ccx_out[1, tl * P:(tl + 1) * P, :])

                # ---- final LN (pure; affine folded into lm head) ----
                with nc.named_scope("final_ln"):
                    for tt in range(NTO):
                        xt_ = xb.tile([P, D], bf16, name="xbrow", tag="xh")
                        ln_pure(h[:, tt, :], xt_[:], tt)
                        for k in range(ND):
                            transpose_to(hT[:, k, tt * P:(tt + 1) * P],
                                         xt_[:, k * P:(k + 1) * P], k)
            # stack pools closed here

            # ---------------- LM head ----------------
            with nc.named_scope("lmhead"):
                lctx = contextlib.ExitStack()
                with lctx:
                    lmw = lctx.enter_context(tc.tile_pool(name="lmw",
                                                          bufs=3))
                    for vc in range(NVC):
                        wch = lmw.tile([P, ND, 512], bf16, name="wlmc")
                        nc.sync.dma_start(out=wch[:], in_=wlm_d[vc])
                        for tt in range(NTO):
                            psl = ps_a.tile([P, 512], f32, name="psq")
                            for k in range(ND):
                                nc.tensor.matmul(
                                    out=psl[:],
                                    lhsT=hT[:, k, tt * P:(tt + 1) * P],
                                    rhs=wch[:, k, :],
                                    start=(k == 0), stop=(k == ND - 1))
                            osb = xb.tile([P, 512], bf16, name="ev512",
                                          tag="ev")
                            if (vc + tt) % 2 == 0:
                                nc.scalar.activation(out=osb[:], in_=psl[:],
                                                     func=AF.Copy)
                            else:
                                nc.vector.tensor_copy(out=osb[:],
                                                      in_=psl[:])
                            nc.sync.dma_start(
                                out=out_d[tt * P:(tt + 1) * P,
                                          vc * 512:(vc + 1) * 512],
                                in_=osb[:])
    nc.compile()
    return nc


_NC_CACHE = {}


def _get_nc(key=(True,)):
    if key not in _NC_CACHE:
        _NC_CACHE[key] = build_nc(collectives=key[0])
    return _NC_CACHE[key]


def kernel(**inputs) -> np.ndarray:
    nc = _get_nc()
    maps, logit_bias = host_prepare(inputs)
    res = run_bass_kernel_spmd(nc, maps, core_ids=list(range(8)),
                               trace=False)
    logits = np.empty((B, T, V + 1), np.float32)
    lb = logit_bias[: V + 1][None, :]
    for c in range(8):
        b_, half = c // 2, c % 2
        logits[b_, half * T2:(half + 1) * T2] = (
            res.results[c]["out"][:, : V + 1].astype(np.float32) + lb)
    return logits


# revision 16
# speedup vs baseline: 1.3144x; 1.0877x over previous
"""BlockDiffusionDecoder (mBART-style 2-layer decoder + BD3LM self-attn mask)
on 8 Trainium2 NeuronCores.

Sharding: sequence-parallel.  Core c owns batch b = c//2 and token half
h = c%2 (512 of the 1024 tokens).  Each core carries the FULL hidden state
in a uniform local layout: local tiles 0-3 = its own token half, local
tiles 4-7 = the global second half (x0 tokens).  For odd cores the two
regions coincide (own half duplicated), which keeps the SPMD program
identical across cores — only input data differs (ids/pos row order and
two 128x128 self-attention mask tiles).

Per layer each core computes K/V for all 8 local tiles but Q, attention,
cross-attention and FFN only for its own 4 tiles; sublayer deltas are
added straight into the fp32 residual (no collectives).  After each
non-final layer a single pair AllGather (1 MB bf16) refreshes local
tiles 4-7.  The LM head is token-parallel: full-vocab weights are
streamed from HBM and each core emits logits for its own 512 tokens
(bf16), so no final AllGather is needed.

Attention computes transposed scores (scores^T[key, query]) directly,
so softmax probabilities come out of the PE already in the layout AV
needs: no per-head transposes.  Additive masks are accumulated into the
score PSUM via identity-matmuls (finite -6e4 for "masked" so 0*mask
stays 0), exp runs straight from PSUM (scores are O(3), no max
subtraction needed), row sums come from ones-vector matmuls, and the
1/sum renormalization is folded into the PSUM->SBUF eviction.

LayerNorm scale/bias are folded into the downstream projection weights /
biases host-side, so on-device LN is just (x - mu) * rsqrt(var + eps).
"""
import sys

if "/opt/trn_rl_repo" not in sys.path:
    sys.path.insert(0, "/opt/trn_rl_repo")

import contextlib

import ml_dtypes
import numpy as np

import concourse.bass as bass
import concourse.bacc as bacc
import concourse.tile as tile
from concourse import mybir
from concourse.bass_utils import run_bass_kernel_spmd
from concourse.masks import make_identity

P = 128
B, D, H, NL, DFF, V, S = 4, 1024, 16, 2, 4096, 32000, 128
T = 1024
T2 = 512             # tokens owned per core
HD = D // H          # 64
BLK = 4
VP = 32768           # padded vocab
NT = 8               # local token tiles (4 own + 4 "x0 region")
NTO = 4              # own token tiles
ND = D // P          # 8 feature tiles
MH = ND              # 8 m-tiles for q/k (2 heads per tile)
NVC = VP // 512      # 64 lm-head 512-vocab chunks
NVG = VP // 1024     # 32 lm-head dma chunks
EMB_SCALE = 32.0     # sqrt(D)
MNEG = -60000.0      # finite "masked" additive value (exp -> 0)
BF = ml_dtypes.bfloat16

f32 = mybir.dt.float32
bf16 = mybir.dt.bfloat16
i32 = mybir.dt.int32
AF = mybir.ActivationFunctionType
ALU = mybir.AluOpType
AX = mybir.AxisListType

PAIRS = [[0, 1], [2, 3], [4, 5], [6, 7]]


def _rhs_tile(w_t: np.ndarray, nchunk: int) -> np.ndarray:
    """[d_in, d_out] -> [n_chunks, 128, k_tiles, nchunk] bf16."""
    d_in, d_out = w_t.shape
    kt = d_in // P
    nc_ = d_out // nchunk
    return np.ascontiguousarray(
        w_t.reshape(kt, P, nc_, nchunk).transpose(2, 1, 0, 3).astype(BF))


def _mask_consts():
    i = np.arange(P)
    diag = np.where((i[:, None] // BLK) == (i[None, :] // BLK), 0.0, MNEG)
    tri_s = np.where((i[:, None] // BLK) > (i[None, :] // BLK), 0.0, MNEG)
    tri_i = np.where((i[:, None] // BLK) >= (i[None, :] // BLK), 0.0, MNEG)
    return (diag.astype(np.float32), tri_s.astype(np.float32),
            tri_i.astype(np.float32))


def _assemble(out_core: np.ndarray) -> np.ndarray:
    """Kernel out [NVC*NTO*P, 512] -> [T2, VP] logits."""
    return (out_core.reshape(NVC, NTO * P, 512)
            .transpose(1, 0, 2).reshape(T2, VP))


def host_prepare(inputs: dict):
    ids = np.asarray(inputs["input_ids"])
    enc = np.asarray(inputs["enc_hidden"], dtype=np.float32)
    emask = np.asarray(inputs["enc_mask"])
    emb = np.ascontiguousarray(np.asarray(inputs["embed_tokens"], np.float32))
    pos = np.asarray(inputs["pos_embed"], np.float32)
    attn_w = np.asarray(inputs["attn_w"], np.float32)
    attn_b = np.asarray(inputs["attn_b"], np.float32)
    ln_w = np.asarray(inputs["ln_w"], np.float32)
    ln_b = np.asarray(inputs["ln_b"], np.float32)
    fc1_w = np.asarray(inputs["fc1_w"], np.float32)
    fc1_b = np.asarray(inputs["fc1_b"], np.float32)
    fc2_w = np.asarray(inputs["fc2_w"], np.float32)
    fc2_b = np.asarray(inputs["fc2_b"], np.float32)
    lm_w = np.asarray(inputs["lm_head_w"], np.float32)
    fs = np.asarray(inputs["final_ln_s"], np.float32)
    fb = np.asarray(inputs["final_ln_b"], np.float32)

    # final LN fold into lm head
    lm_pad = np.zeros((VP, D), np.float32)
    lm_pad[: V + 1] = lm_w
    lm_t = lm_pad.T * fs[:, None]                      # [D, VP]
    logit_bias = fb @ lm_pad.T                         # [VP] host-added
    wlm = np.ascontiguousarray(
        lm_t.reshape(ND, P, NVG, 1024).transpose(2, 1, 0, 3).astype(BF))

    mdiag, mtris, mtrii = _mask_consts()
    allneg = np.full((P, P), MNEG, np.float32)

    # shared (core-independent) weight tensors
    shared = {"emb": emb, "wlm": wlm,
              "lnemb": np.stack([np.asarray(inputs["ln_emb_s"], np.float32),
                                 np.asarray(inputs["ln_emb_b"], np.float32)])}
    for l in range(NL):
        for a, tag in ((0, "s"), (1, "c")):
            wq, wk, wv, wo = attn_w[l, a]
            bq, bk, bv, bo = attn_b[l, a]
            s_ln = ln_w[l, a]
            b_ln = ln_b[l, a]
            # fold LN affine into x-side projections (q always; k,v only
            # for self-attn where they read the LN'd x)
            wqT = wq.T * s_ln[:, None]
            bq_e = bq + b_ln @ wq.T
            if a == 0:
                wkT = wk.T * s_ln[:, None]
                bk_e = bk + b_ln @ wk.T
                wvT = wv.T * s_ln[:, None]
                bv_e = bv + b_ln @ wv.T
            else:
                wkT, bk_e, wvT, bv_e = wk.T, bk, wv.T, bv
            shared[f"wq{tag}{l}"] = _rhs_tile(wqT, P)
            shared[f"wk{tag}{l}"] = _rhs_tile(wkT, P if a == 0 else 512)
            shared[f"wv{tag}{l}"] = _rhs_tile(wvT, 512)
            shared[f"wo{tag}{l}"] = _rhs_tile(wo.T, 512)
            shared[f"bq{tag}{l}"] = np.ascontiguousarray(
                bq_e.reshape(MH, P).T)
            shared[f"bk{tag}{l}"] = np.ascontiguousarray(
                bk_e.reshape(MH, P).T)
            shared[f"bv{tag}{l}"] = bv_e.reshape(1, D).copy()
            shared[f"bo{tag}{l}"] = bo.reshape(1, D).copy()
        s3, b3 = ln_w[l, 2], ln_b[l, 2]
        f1T = fc1_w[l].T * s3[:, None]
        bf1_e = fc1_b[l] + b3 @ fc1_w[l].T
        shared[f"wf1{l}"] = _rhs_tile(f1T, P)
        shared[f"bf1{l}"] = np.ascontiguousarray(
            bf1_e.reshape(DFF // P, P).T)
        # fc2 streamed as [kk-group of 8][n2] chunks: [4, 2, P, 8, 512]
        shared[f"wf2{l}"] = np.ascontiguousarray(
            fc2_w[l].T.reshape(4, 8, P, 2, 512)
            .transpose(0, 3, 2, 1, 4).astype(BF))
        shared[f"bf2{l}"] = fc2_b[l].reshape(1, D).copy()

    maps = []
    for c in range(8):
        b_, half = c // 2, c % 2
        own = slice(half * T2, half * T2 + T2)
        x0 = slice(T2, T)
        m = dict(shared)
        m["ids"] = np.concatenate(
            [ids[b_, own], ids[b_, x0]]).reshape(T, 1).astype(np.int32)
        m["pos"] = np.ascontiguousarray(
            np.concatenate([pos[own], pos[x0]], axis=0))
        m["encT"] = np.ascontiguousarray(enc[b_].T.astype(BF))
        m["cmaskT"] = np.ascontiguousarray(
            ((1.0 - emask[b_].astype(np.float32)) * MNEG).reshape(S, 1))
        mA = mdiag if half == 0 else mtrii
        mB = mtris if half == 0 else allneg
        m["mAT"] = np.ascontiguousarray(mA.T.astype(BF))
        m["mBT"] = np.ascontiguousarray(mB.T.astype(BF))
        maps.append(m)
    return maps, logit_bias


def build_nc(collectives=True, gelu=AF.Gelu_apprx_tanh):
    nc = bacc.Bacc(num_devices=8 if collectives else None, trn_type="TRN2")

    ids_d = nc.dram_tensor("ids", [T, 1], i32, kind="ExternalInput")
    emb_d = nc.dram_tensor("emb", [V + 1, D], f32, kind="ExternalInput")
    pos_d = nc.dram_tensor("pos", [T, D], f32, kind="ExternalInput")
    encT_d = nc.dram_tensor("encT", [D, S], bf16, kind="ExternalInput")
    cmaskT_d = nc.dram_tensor("cmaskT", [S, 1], f32, kind="ExternalInput")
    lnemb_d = nc.dram_tensor("lnemb", [2, D], f32, kind="ExternalInput")
    mAT_d = nc.dram_tensor("mAT", [P, P], bf16, kind="ExternalInput")
    mBT_d = nc.dram_tensor("mBT", [P, P], bf16, kind="ExternalInput")
    wlm_d = nc.dram_tensor("wlm", [NVG, P, ND, 1024], bf16,
                           kind="ExternalInput")
    wd, bd = {}, {}
    for l in range(NL):
        for tg in ("s", "c"):
            wd[f"wq{tg}{l}"] = nc.dram_tensor(
                f"wq{tg}{l}", [MH, P, ND, P], bf16, kind="ExternalInput")
            wk_shape = ([MH, P, ND, P] if tg == "s"
                        else [2, P, ND, 512])
            wd[f"wk{tg}{l}"] = nc.dram_tensor(
                f"wk{tg}{l}", wk_shape, bf16, kind="ExternalInput")
            wd[f"wv{tg}{l}"] = nc.dram_tensor(
                f"wv{tg}{l}", [2, P, ND, 512], bf16, kind="ExternalInput")
            wd[f"wo{tg}{l}"] = nc.dram_tensor(
                f"wo{tg}{l}", [2, P, ND, 512], bf16, kind="ExternalInput")
            bd[f"bq{tg}{l}"] = nc.dram_tensor(
                f"bq{tg}{l}", [P, MH], f32, kind="ExternalInput")
            bd[f"bk{tg}{l}"] = nc.dram_tensor(
                f"bk{tg}{l}", [P, MH], f32, kind="ExternalInput")
            bd[f"bv{tg}{l}"] = nc.dram_tensor(
                f"bv{tg}{l}", [1, D], f32, kind="ExternalInput")
            bd[f"bo{tg}{l}"] = nc.dram_tensor(
                f"bo{tg}{l}", [1, D], f32, kind="ExternalInput")
        wd[f"wf1{l}"] = nc.dram_tensor(
            f"wf1{l}", [DFF // P, P, ND, P], bf16, kind="ExternalInput")
        bd[f"bf1{l}"] = nc.dram_tensor(
            f"bf1{l}", [P, DFF // P], f32, kind="ExternalInput")
        wd[f"wf2{l}"] = nc.dram_tensor(
            f"wf2{l}", [4, 2, P, 8, 512], bf16, kind="ExternalInput")
        bd[f"bf2{l}"] = nc.dram_tensor(
            f"bf2{l}", [1, D], f32, kind="ExternalInput")
    out_d = nc.dram_tensor("out", [NVC * NTO * P, 512], bf16,
                           kind="ExternalOutput")

    def bcast(ap_1d, p=P):
        return bass.AP(tensor=ap_1d.tensor, offset=ap_1d.offset,
                       ap=[[0, p]] + list(ap_1d.ap))

    with tile.TileContext(nc) as tc:
        gctx = contextlib.ExitStack()
        with gctx:
            consts = gctx.enter_context(tc.tile_pool(name="consts", bufs=1))
            small = gctx.enter_context(tc.tile_pool(name="small", bufs=4))
            sp = gctx.enter_context(tc.tile_pool(name="sp", bufs=2))
            xb = gctx.enter_context(tc.tile_pool(name="xb", bufs=2))
            dram = gctx.enter_context(
                tc.tile_pool(name="dram", bufs=1, space="DRAM"))
            ps_a = gctx.enter_context(
                tc.tile_pool(name="ps_a", bufs=4, space="PSUM"))
            ps_av = gctx.enter_context(
                tc.tile_pool(name="ps_av", bufs=2, space="PSUM"))
            ps_tr = gctx.enter_context(
                tc.tile_pool(name="ps_tr", bufs=2, space="PSUM"))
            glob = gctx.enter_context(tc.tile_pool(name="glob", bufs=1))

            ident = consts.tile([P, P], bf16)
            make_identity(nc, ident[:])
            eps_t = consts.tile([P, 1], f32)
            nc.vector.memset(eps_t[:], 1e-5)
            ones_t = consts.tile([P, 64], bf16)
            nc.vector.memset(ones_t[:], 1.0)
            mAT = consts.tile([P, P], bf16)
            nc.sync.dma_start(out=mAT[:], in_=mAT_d[:])
            mBT = consts.tile([P, P], bf16)
            nc.sync.dma_start(out=mBT[:], in_=mBT_d[:])
            cmaskT = consts.tile([S, 1], f32)
            nc.sync.dma_start(out=cmaskT[:], in_=cmaskT_d[:])

            # final hidden (transposed) for the LM head — outlives the
            # stack pools
            hT = glob.tile([P, ND, T2], bf16)

            ccx_in = dram.tile([T2, D], bf16, name="ccx_in")
            ccx_out = (dram.tile([2, T2, D], bf16, name="ccx_out")
                       if collectives else None)

            def ln_stats(src_ap):
                """-> (negmur [P,1], rstd [P,1]): x_hat = x*rstd + negmur."""
                st = small.tile([P, 2, 6], f32, name="lnstats")
                nc.vector.bn_stats(out=st[:, 0, :], in_=src_ap[:, 0:512])
                nc.vector.bn_stats(out=st[:, 1, :], in_=src_ap[:, 512:1024])
                mv = small.tile([P, 2], f32, name="lnmv")
                nc.vector.bn_aggr(out=mv[:], in_=st[:])
                rstd = small.tile([P, 1], f32, name="lnrstd")
                nc.scalar.activation(out=rstd[:], in_=mv[:, 1:2],
                                     func=AF.Sqrt, bias=eps_t[:])
                nc.vector.reciprocal(out=rstd[:], in_=rstd[:])
                negmur = small.tile([P, 1], f32, name="lnnm")
                nc.vector.tensor_scalar(out=negmur[:], in0=mv[:, 0:1],
                                        scalar1=rstd[:], scalar2=-1.0,
                                        op0=ALU.mult, op1=ALU.mult)
                return negmur, rstd

            def ln_pure(src_ap, dst_ap, par):
                """dst = (src - mean) * rsqrt(var+eps); engine by parity."""
                negmur, rstd = ln_stats(src_ap)
                if par % 2 == 0:
                    nc.scalar.activation(out=dst_ap, in_=src_ap,
                                         func=AF.Identity,
                                         scale=rstd[:], bias=negmur[:])
                else:
                    nc.vector.tensor_scalar(out=dst_ap, in0=src_ap,
                                            scalar1=rstd[:],
                                            scalar2=negmur[:],
                                            op0=ALU.mult, op1=ALU.add)

            # ---------------- stack phase ----------------
            sctx = contextlib.ExitStack()
            with sctx:
                pers = sctx.enter_context(tc.tile_pool(name="pers", bufs=1))
                wp = sctx.enter_context(tc.tile_pool(name="wp", bufs=2))
                lnp = sctx.enter_context(tc.tile_pool(name="lnp", bufs=1))

                h = pers.tile([P, NT, D], f32)
                encT = pers.tile([P, ND, S], bf16)
                nc.sync.dma_start(
                    out=encT[:],
                    in_=encT_d.rearrange("(k p) s -> p k s", p=P))
                # cross-attn K/V for both layers, precomputed
                encKT = pers.tile([P, NL, MH, S], bf16)
                encV = pers.tile([P, NL, D], bf16)

                def bias_bcast(src_row, name):
                    t = lnp.tile([P, D], bf16, name=name, tag="bb", bufs=2)
                    nc.gpsimd.dma_start(out=t[:], in_=bcast(src_row))
                    return t

                def transpose_to(dst_ap, src_ap, par):
                    tp_ = ps_tr.tile([P, P], bf16, name="trps")
                    nc.tensor.transpose(tp_[:], src_ap, ident[:])
                    if par % 2 == 0:
                        nc.scalar.activation(out=dst_ap, in_=tp_[:],
                                             func=AF.Copy)
                    else:
                        nc.vector.tensor_copy(out=dst_ap, in_=tp_[:])

                # ---- embed + emb LN (general affine) ----
                with nc.named_scope("embed"):
                    lnes = bias_bcast(lnemb_d[0], "lnes")
                    lneb = bias_bcast(lnemb_d[1], "lneb")
                    for tt in range(NT):
                        idt = small.tile([P, 1], i32, name="idt")
                        nc.sync.dma_start(out=idt[:],
                                          in_=ids_d[tt * P:(tt + 1) * P])
                        g = xb.tile([P, D], f32, name="xrow", tag="xf",
                                    bufs=4)
                        nc.gpsimd.indirect_dma_start(
                            out=g[:], out_offset=None, in_=emb_d[:],
                            in_offset=bass.IndirectOffsetOnAxis(
                                ap=idt[:, :1], axis=0))
                        pt = xb.tile([P, D], f32, name="xrow2", tag="xf",
                                     bufs=4)
                        nc.sync.dma_start(out=pt[:],
                                          in_=pos_d[tt * P:(tt + 1) * P])
                        nc.vector.tensor_scalar(out=g[:], in0=g[:],
                                                scalar1=EMB_SCALE,
                                                scalar2=None, op0=ALU.mult)
                        nc.vector.tensor_tensor(out=g[:], in0=g[:],
                                                in1=pt[:], op=ALU.add)
                        negmur, rstd = ln_stats(g[:])
                        nc.scalar.activation(out=pt[:], in_=g[:],
                                             func=AF.Identity,
                                             scale=rstd[:],
                                             bias=negmur[:])
                        nc.vector.tensor_tensor(out=pt[:], in0=pt[:],
                                                in1=lnes[:], op=ALU.mult)
                        nc.vector.tensor_tensor(out=h[:, tt, :], in0=pt[:],
                                                in1=lneb[:], op=ALU.add)

                    # enc K/V for both layers (independent of h)
                    for l in range(NL):
                        bk2 = small.tile([P, MH], f32, name="bk2")
                        nc.sync.dma_start(out=bk2[:], in_=bd[f"bkc{l}"][:])
                        bv2 = bias_bcast(bd[f"bvc{l}"][0], "bv2")
                        kraw = sp.tile([S, D], bf16, name="kraw", bufs=1)
                        for hf in range(2):
                            wck = wp.tile([P, ND, 512], bf16, name="wch",
                                          tag="wch")
                            nc.sync.dma_start(out=wck[:],
                                              in_=wd[f"wkc{l}"][hf])
                            psk = ps_a.tile([P, 512], f32, name="psq")
                            for k in range(ND):
                                nc.tensor.matmul(
                                    out=psk[:], lhsT=encT[:, k, :],
                                    rhs=wck[:, k, :],
                                    start=(k == 0), stop=(k == ND - 1))
                            nc.scalar.activation(
                                out=kraw[:, hf * 512:(hf + 1) * 512],
                                in_=psk[:], func=AF.Copy)
                        for mq in range(MH):
                            tpk = ps_tr.tile([P, P], bf16, name="trps")
                            nc.tensor.transpose(
                                tpk[:], kraw[:, mq * P:(mq + 1) * P],
                                ident[:])
                            nc.vector.tensor_scalar(
                                out=encKT[:, l, mq, :], in0=tpk[:],
                                scalar1=bk2[:, mq:mq + 1], scalar2=None,
                                op0=ALU.add)
                        for hf in range(2):
                            wcv = wp.tile([P, ND, 512], bf16, name="wch",
                                          tag="wch")
                            nc.sync.dma_start(out=wcv[:],
                                              in_=wd[f"wvc{l}"][hf])
                            psv = ps_a.tile([P, 512], f32, name="psq")
                            for k in range(ND):
                                nc.tensor.matmul(
                                    out=psv[:], lhsT=encT[:, k, :],
                                    rhs=wcv[:, k, :],
                                    start=(k == 0), stop=(k == ND - 1))
                            nc.vector.tensor_tensor(
                                out=encV[:, l, hf * 512:(hf + 1) * 512],
                                in0=psv[:],
                                in1=bv2[:, hf * 512:(hf + 1) * 512],
                                op=ALU.add)

                def proj_qk(dst, w_key, b_sb, scale, src_xT, ncols):
                    """dst[:, m, 0:ncols] = (x @ W)^T + b, x = src_xT cols."""
                    for m in range(MH):
                        wch = wp.tile([P, ND, P], bf16, name="wch", tag="wch")
                        nc.sync.dma_start(out=wch[:], in_=wd[w_key][m])
                        for cb in range(ncols // 512):
                            psq = ps_a.tile([P, 512], f32, name="psq")
                            for k in range(ND):
                                nc.tensor.matmul(
                                    out=psq[:], lhsT=wch[:, k, :],
                                    rhs=src_xT[:, k,
                                               cb * 512:(cb + 1) * 512],
                                    start=(k == 0), stop=(k == ND - 1))
                            if scale is None:
                                nc.vector.tensor_scalar(
                                    out=dst[:, m, cb * 512:(cb + 1) * 512],
                                    in0=psq[:], scalar1=b_sb[:, m:m + 1],
                                    scalar2=None, op0=ALU.add)
                            else:
                                nc.vector.tensor_scalar(
                                    out=dst[:, m, cb * 512:(cb + 1) * 512],
                                    in0=psq[:], scalar1=b_sb[:, m:m + 1],
                                    scalar2=scale, op0=ALU.add, op1=ALU.mult)

                def oproj_update(src_oT, wo_key, bo_key):
                    """h[own] += oT @ Wo + bo (bias via pre-add)."""
                    bo_b = bias_bcast(bd[bo_key][0], "bo_b")
                    for tt in range(NTO):
                        nc.vector.tensor_tensor(out=h[:, tt, :],
                                                in0=h[:, tt, :],
                                                in1=bo_b[:], op=ALU.add)
                    for n2 in range(2):
                        wch = wp.tile([P, ND, 512], bf16, name="wch",
                                      tag="wch")
                        nc.sync.dma_start(out=wch[:], in_=wd[wo_key][n2])
                        for tt in range(NTO):
                            pso = ps_a.tile([P, 512], f32, name="psq")
                            for k in range(ND):
                                nc.tensor.matmul(
                                    out=pso[:],
                                    lhsT=src_oT[:, k, tt * P:(tt + 1) * P],
                                    rhs=wch[:, k, :],
                                    start=(k == 0), stop=(k == ND - 1))
                            nc.vector.tensor_tensor(
                                out=h[:, tt, n2 * 512:(n2 + 1) * 512],
                                in0=h[:, tt, n2 * 512:(n2 + 1) * 512],
                                in1=pso[:], op=ALU.add)

                for l in range(NL):
                  with nc.named_scope(f"layer{l}"):
                    # ======== self attention ========
                    xT = pers.tile([P, ND, T], bf16, name="xT", tag="xT")
                    for tt in range(NT):
                        xt_ = xb.tile([P, D], bf16, name="xbrow", tag="xh")
                        ln_pure(h[:, tt, :], xt_[:], tt)
                        for k in range(ND):
                            transpose_to(xT[:, k, tt * P:(tt + 1) * P],
                                         xt_[:, k * P:(k + 1) * P], k)

                    qT = pers.tile([P, MH, T2], bf16, name="qT", tag="qT")
                    kT = pers.tile([P, MH, T], bf16, name="kT", tag="kT")
                    vv = pers.tile([P, NT, D], bf16, name="vv", tag="vv")
                    bq_sb = small.tile([P, MH], f32, name="bq")
                    nc.sync.dma_start(out=bq_sb[:], in_=bd[f"bqs{l}"][:])
                    bk_sb = small.tile([P, MH], f32, name="bk")
                    nc.sync.dma_start(out=bk_sb[:], in_=bd[f"bks{l}"][:])
                    bv_b = bias_bcast(bd[f"bvs{l}"][0], "bv_b")

                    proj_qk(qT, f"wqs{l}", bq_sb, 0.125, xT, T2)
                    proj_qk(kT, f"wks{l}", bk_sb, None, xT, T)
                    for hf in range(2):
                        wch_v = wp.tile([P, ND, 512], bf16, name="wch",
                                        tag="wch")
                        nc.sync.dma_start(out=wch_v[:], in_=wd[f"wvs{l}"][hf])
                        for tt in range(NT):
                            psv = ps_a.tile([P, 512], f32, name="psq")
                            for k in range(ND):
                                nc.tensor.matmul(
                                    out=psv[:],
                                    lhsT=xT[:, k, tt * P:(tt + 1) * P],
                                    rhs=wch_v[:, k, :],
                                    start=(k == 0), stop=(k == ND - 1))
                            nc.vector.tensor_tensor(
                                out=vv[:, tt, hf * 512:(hf + 1) * 512],
                                in0=psv[:],
                                in1=bv_b[:, hf * 512:(hf + 1) * 512],
                                op=ALU.add)

                    # transposed scores + softmax + AV per head
                    oT = pers.tile([P, ND, T2], bf16, name="oT", tag="oT")
                    for hl in range(H):
                        prow = slice((hl % 2) * 64, (hl % 2) * 64 + 64)
                        mq = hl // 2
                        hds = slice(hl * HD, (hl + 1) * HD)
                        ptile = sp.tile([P, 5, T2], bf16, name="ptile")
                        # diag tiles (slot 4): scoresT quarters + mask
                        psD = ps_a.tile([P, 512], f32, name="psq")
                        for qi in range(NTO):
                            cs = slice(qi * P, (qi + 1) * P)
                            nc.tensor.matmul(
                                out=psD[:, cs],
                                lhsT=kT[prow, mq, qi * P:(qi + 1) * P],
                                rhs=qT[prow, mq, qi * P:(qi + 1) * P],
                                start=(qi == 0), stop=False)
                            nc.tensor.matmul(
                                out=psD[:, cs], lhsT=ident[:], rhs=mAT[:],
                                start=False, stop=(qi == NTO - 1))
                        nc.scalar.activation(out=ptile[:, 4, :], in_=psD[:],
                                             func=AF.Exp)
                        # strip tiles (slots 0-3): key tile 4+j covers
                        # queries j..3; the leading block of each strip
                        # (query tile j) is masked with mBT.  Strips 0,1
                        # get their own bank; strips 2 (256 wide) and 3
                        # (128 wide) pack into one bank at offsets 0/256.
                        for j in range(2):
                            w_ = (NTO - j) * P
                            psS = ps_a.tile([P, 512], f32, name="psq")
                            nc.tensor.matmul(
                                out=psS[:, 0:w_],
                                lhsT=kT[prow, mq, (4 + j) * P:(5 + j) * P],
                                rhs=qT[prow, mq, j * P:T2],
                                start=True, stop=False)
                            nc.tensor.matmul(
                                out=psS[:, 0:P], lhsT=ident[:],
                                rhs=mBT[:], start=False, stop=True)
                            nc.scalar.activation(
                                out=ptile[:, j, j * P:T2],
                                in_=psS[:, 0:w_], func=AF.Exp)
                        psS = ps_a.tile([P, 512], f32, name="psq")
                        for j, off in ((2, 0), (3, 256)):
                            w_ = (NTO - j) * P
                            nc.tensor.matmul(
                                out=psS[:, off:off + w_],
                                lhsT=kT[prow, mq, (4 + j) * P:(5 + j) * P],
                                rhs=qT[prow, mq, j * P:T2],
                                start=(j == 2), stop=False)
                            nc.tensor.matmul(
                                out=psS[:, off:off + P], lhsT=ident[:],
                                rhs=mBT[:], start=False, stop=(j == 3))
                        for j, off in ((2, 0), (3, 256)):
                            w_ = (NTO - j) * P
                            nc.scalar.activation(
                                out=ptile[:, j, j * P:T2],
                                in_=psS[:, off:off + w_], func=AF.Exp)

                        sums_ps = ps_a.tile([P, 512], f32, name="psq")
                        nc.tensor.matmul(out=sums_ps[0:1, :],
                                         lhsT=ones_t[:, 0:1],
                                         rhs=ptile[:, 4, :],
                                         start=True, stop=False)
                        for j in range(NTO):
                            nc.tensor.matmul(
                                out=sums_ps[0:1, j * P:T2],
                                lhsT=ones_t[:, 0:1],
                                rhs=ptile[:, j, j * P:T2],
                                start=False, stop=(j == NTO - 1))
                        recip_sb = small.tile([1, 512], bf16, name="recip",
                                              bufs=2)
                        with nc.allow_low_precision(
                                reason="softmax 1/sum in bf16"):
                            nc.vector.reciprocal(out=recip_sb[:],
                                                 in_=sums_ps[0:1, :])
                        rb_ps = ps_av.tile([64, 512], f32, name="pav")
                        nc.tensor.matmul(out=rb_ps[:], lhsT=ones_t[0:1, :],
                                         rhs=recip_sb[:], start=True,
                                         stop=True)
                        recip_b = xb.tile([64, 512], bf16, name="rbb",
                                          tag="rb")
                        nc.scalar.activation(out=recip_b[:], in_=rb_ps[:],
                                             func=AF.Copy)
                        pav = ps_av.tile([64, T2], f32, name="pav")
                        for qi in range(NTO):
                            nc.tensor.matmul(
                                out=pav[:, qi * P:(qi + 1) * P],
                                lhsT=vv[:, qi, hds],
                                rhs=ptile[:, 4, qi * P:(qi + 1) * P],
                                start=(qi == 0), stop=False)
                        for j in range(NTO):
                            nc.tensor.matmul(
                                out=pav[:, j * P:T2],
                                lhsT=vv[:, 4 + j, hds],
                                rhs=ptile[:, j, j * P:T2],
                                start=False, stop=(j == NTO - 1))
                        nc.vector.tensor_tensor(out=oT[prow, mq, :],
                                                in0=pav[:], in1=recip_b[:],
                                                op=ALU.mult)
                    oproj_update(oT, f"wos{l}", f"bos{l}")

                    # ======== cross attention ========
                    x2T = pers.tile([P, ND, T2], bf16, name="x2T", tag="x2T")
                    for ti in range(NTO):
                        xt_ = xb.tile([P, D], bf16, name="xbrow", tag="xh")
                        ln_pure(h[:, ti, :], xt_[:], ti)
                        for k in range(ND):
                            transpose_to(x2T[:, k, ti * P:(ti + 1) * P],
                                         xt_[:, k * P:(k + 1) * P], k)
                    q2T = pers.tile([P, MH, T2], bf16, name="q2T", tag="qT")
                    bq2_sb = small.tile([P, MH], f32, name="bq2")
                    nc.sync.dma_start(out=bq2_sb[:], in_=bd[f"bqc{l}"][:])
                    proj_qk(q2T, f"wqc{l}", bq2_sb, 0.125, x2T, T2)

                    o2T = pers.tile([P, ND, T2], bf16, name="o2T", tag="oT")
                    for hl in range(H):
                        prow = slice((hl % 2) * 64, (hl % 2) * 64 + 64)
                        mq = hl // 2
                        hds = slice(hl * HD, (hl + 1) * HD)
                        p2tile = sp.tile([S, T2], bf16, name="p2tile")
                        ps2 = ps_a.tile([P, 512], f32, name="psq")
                        nc.tensor.matmul(out=ps2[:],
                                         lhsT=encKT[prow, l, mq, :],
                                         rhs=q2T[prow, mq, :],
                                         start=True, stop=True)
                        nc.scalar.activation(out=p2tile[:], in_=ps2[:],
                                             func=AF.Exp, bias=cmaskT[:])
                        sums_ps = ps_a.tile([P, 512], f32, name="psq")
                        nc.tensor.matmul(out=sums_ps[0:1, :],
                                         lhsT=ones_t[:, 0:1],
                                         rhs=p2tile[:], start=True,
                                         stop=True)
                        recip_sb = small.tile([1, 512], bf16, name="recip",
                                              bufs=2)
                        with nc.allow_low_precision(
                                reason="softmax 1/sum in bf16"):
                            nc.vector.reciprocal(out=recip_sb[:],
                                                 in_=sums_ps[0:1, :])
                        rb_ps = ps_av.tile([64, 512], f32, name="pav")
                        nc.tensor.matmul(out=rb_ps[:], lhsT=ones_t[0:1, :],
                                         rhs=recip_sb[:], start=True,
                                         stop=True)
                        recip_b = xb.tile([64, 512], bf16, name="rbb",
                                          tag="rb")
                        nc.scalar.activation(out=recip_b[:], in_=rb_ps[:],
                                             func=AF.Copy)
                        pav2 = ps_av.tile([64, T2], f32, name="pav")
                        nc.tensor.matmul(out=pav2[:], lhsT=encV[:, l, hds],
                                         rhs=p2tile[:], start=True,
                                         stop=True)
                        nc.vector.tensor_tensor(out=o2T[prow, mq, :],
                                                in0=pav2[:], in1=recip_b[:],
                                                op=ALU.mult)
                    oproj_update(o2T, f"woc{l}", f"boc{l}")

                    # ======== FFN ========
                    x3T = pers.tile([P, ND, T2], bf16, name="x3T", tag="x2T")
                    for ti in range(NTO):
                        xt_ = xb.tile([P, D], bf16, name="xbrow", tag="xh")
                        ln_pure(h[:, ti, :], xt_[:], ti)
                        for k in range(ND):
                            transpose_to(x3T[:, k, ti * P:(ti + 1) * P],
                                         xt_[:, k * P:(k + 1) * P], k)
                    bf1_sb = small.tile([P, DFF // P], f32, name="bf1s")
                    nc.sync.dma_start(out=bf1_sb[:], in_=bd[f"bf1{l}"][:])
                    bf2_b = bias_bcast(bd[f"bf2{l}"][0], "bf2_b")
                    for tt in range(NTO):
                        nc.vector.tensor_tensor(out=h[:, tt, :],
                                                in0=h[:, tt, :],
                                                in1=bf2_b[:], op=ALU.add)
                    for dh in range(2):
                        gT = pers.tile([P, 16, T2], bf16, name="gT",
                                       tag="xT")
                        for df in range(16):
                            dff = dh * 16 + df
                            f1c = wp.tile([P, ND, P], bf16, name="wch",
                                          tag="wch")
                            nc.sync.dma_start(out=f1c[:],
                                              in_=wd[f"wf1{l}"][dff])
                            psf = ps_a.tile([P, 512], f32, name="psq")
                            for k in range(ND):
                                nc.tensor.matmul(
                                    out=psf[:], lhsT=f1c[:, k, :],
                                    rhs=x3T[:, k, :],
                                    start=(k == 0), stop=(k == ND - 1))
                            nc.scalar.activation(
                                out=gT[:, df, :], in_=psf[:], func=gelu,
                                bias=bf1_sb[:, dff:dff + 1])
                        for n2 in range(2):
                            psf2s = [ps_a.tile([P, 512], f32, name="psq")
                                     for _ in range(NTO)]
                            for kg in range(2):
                                f2c = wp.tile([P, 8, 512], bf16, name="wch",
                                              tag="wch")
                                nc.sync.dma_start(
                                    out=f2c[:],
                                    in_=wd[f"wf2{l}"][dh * 2 + kg, n2])
                                for tt in range(NTO):
                                    for kk in range(8):
                                        nc.tensor.matmul(
                                            out=psf2s[tt][:],
                                            lhsT=gT[:, kg * 8 + kk,
                                                    tt * P:(tt + 1) * P],
                                            rhs=f2c[:, kk, :],
                                            start=(kg == 0 and kk == 0),
                                            stop=(kg == 1 and kk == 7))
                            for tt in range(NTO):
                                nc.vector.tensor_tensor(
                                    out=h[:, tt, n2 * 512:(n2 + 1) * 512],
                                    in0=h[:, tt, n2 * 512:(n2 + 1) * 512],
                                    in1=psf2s[tt][:], op=ALU.add)

                    # ======== pair exchange (not after last layer) ========
                    if l < NL - 1 and collectives:
                        for tl in range(NTO):
                            nc.gpsimd.dma_start(
                                out=ccx_in[tl * P:(tl + 1) * P, :],
                                in_=h[:, tl, :])
                        nc.gpsimd.collective_compute(
                            "AllGather", ALU.bypass, replica_groups=PAIRS,
                            ins=[ccx_in[:]], outs=[ccx_out[:]])
                        for tl in range(NTO):
                            nc.gpsimd.dma_start(
                                out=h[:, 4 + tl, :],
                                in_=ccx_out[1, tl * P:(tl + 1) * P, :])

                # ---- final LN (pure; affine folded into lm head) ----
                with nc.named_scope("final_ln"):
                    for tt in range(NTO):
                        xt_ = xb.tile([P, D], bf16, name="xbrow", tag="xh")
                        ln_pure(h[:, tt, :], xt_[:], tt)
                        for k in range(ND):
                            transpose_to(hT[:, k, tt * P:(tt + 1) * P],
                                         xt_[:, k * P:(k + 1) * P], k)
            # stack pools closed here

            # ---------------- LM head ----------------
            with nc.named_scope("lmhead"):
                lctx = contextlib.ExitStack()
                with lctx:
                    lmw = lctx.enter_context(tc.tile_pool(name="lmw",
                                                          bufs=3))
                    for vg in range(NVG):
                        wch = lmw.tile([P, ND, 1024], bf16, name="wlmc")
                        nc.sync.dma_start(out=wch[:], in_=wlm_d[vg])
                        for tt in range(NTO):
                            for hf in range(2):
                                vc = vg * 2 + hf
                                psl = ps_a.tile([P, 512], f32, name="psq")
                                for k in range(ND):
                                    nc.tensor.matmul(
                                        out=psl[:],
                                        lhsT=hT[:, k, tt * P:(tt + 1) * P],
                                        rhs=wch[:, k,
                                                hf * 512:(hf + 1) * 512],
                                        start=(k == 0), stop=(k == ND - 1))
                                osb = xb.tile([P, 512], bf16, name="ev512",
                                              tag="ev", bufs=6)
                                if (vc + tt) % 2 == 0:
                                    nc.scalar.activation(out=osb[:],
                                                         in_=psl[:],
                                                         func=AF.Copy)
                                else:
                                    nc.vector.tensor_copy(out=osb[:],
                                                          in_=psl[:])
                                ro = (vc * NTO + tt) * P
                                nc.sync.dma_start(
                                    out=out_d[ro:ro + P, :], in_=osb[:])
    nc.compile()
    return nc


_NC_CACHE = {}


def _get_nc(key=(True,)):
    if key not in _NC_CACHE:
        _NC_CACHE[key] = build_nc(collectives=key[0])
    return _NC_CACHE[key]


def kernel(**inputs) -> np.ndarray:
    nc = _get_nc()
    maps, logit_bias = host_prepare(inputs)
    res = run_bass_kernel_spmd(nc, maps, core_ids=list(range(8)),
                               trace=False)
    logits = np.empty((B, T, V + 1), np.float32)
    lb = logit_bias[: V + 1][None, :]
    for c in range(8):
        b_, half = c // 2, c % 2
        full = _assemble(res.results[c]["out"])
        logits[b_, half * T2:(half + 1) * T2] = (
            full[:, : V + 1].astype(np.float32) + lb)
    return logits
